# revision 5
# baseline (speedup 1.0000x reference)
"""Trainium2 Bass kernel for nn_DotProductAttention (SQ=SK=2048, B=2, NP=32, HN=64).

v3 design (8 NeuronCores, batch*heads sharded, 8 heads per core = 4 pairs):

  - S^T tiles [128 k, 2 heads, <=512 s] per (k-tile, sq-block) in PSUM.
    QK matmul: lhsT = K^T chunk (head A on partitions 0-63, head B on 64-127),
    rhs = Q^T bf16 for j<2; fp8 DoubleRow (33x2 rows incl. bias row) for j>=2.
  - The exp-arg scale is folded into Q on the host so PSUM holds
    Z = (2^m/ln2) * (S/8)  (m = mantissa bits of the prob dtype: 7 for bf16,
    3 for fp8e4m3; the fp8 path also adds +BR8 via its matmul bias row).
  - exp runs on TWO engines, statically load-balanced:
      ACT: true exp via activation(Exp, scale[, bias]) from PSUM.
      DVE: Schraudolph bit-trick: one tensor_scalar(add B, max 0) converts
           Z to int16/int8 (C-cast truncation; +0.5 folded into B) whose raw
           bits ARE bf16/fp8 probs (written through a bitcast view of the
           pp tile). max(.,0) clamps masked/underflowed scores to +0.0.
  - Causal mask: constant strictly-upper-triangular Tm (-16000) accumulated
    into diagonal-tile PSUM by one extra matmul; both exp paths then emit
    exactly 0. Diagonal tiles only compute s >= 128*t (live extent).
  - PV with pp stationary: out[128 s, 65] += pp_chunk^T @ V_aug over k-tiles;
    V_aug has a ones column so row 64 accumulates the softmax denominator.
    fp8 pp uses DoubleRow (two k-tiles per op, 0.5 cyc/col).
  - NO on-device normalize: the [128, 4, 65] accumulators are evacuated
    PSUM->SBUF as bf16 (one copy per head, on whichever exp engine the
    balancer picks) and DMA'd out; the host divides by the denominator.

The walrus build in this container only accepts ONE sync-wait per
instruction; split_multiwaits() rewrites the Tile-scheduled program.
"""

import math

import numpy as np

SQ, SK, B, NP, HN = 2048, 2048, 2, 32, 64
NCORES = 8
HPC = B * NP // NCORES          # heads per core = 8
PAIRS = HPC // 2                # 4
P = 128
SQ_BLK = 512
NBLK = SQ // SQ_BLK             # 4
SKT = SK // P                   # 16
VF = HN + 1                     # 65: V columns + ones column (denominator)
FP8_FROM = 2                    # first sq-block computed in fp8 + DoubleRow

# --- Schraudolph constants -------------------------------------------------
LN2 = math.log(2.0)
A16 = 128.0 / (8.0 * LN2)       # fold into bf16-path q: Z16 = A16 * S_raw
A8 = 8.0 / (8.0 * LN2)          # fold into fp8-path q:  Z8  = A8 * S_raw (+BR8)
BR8 = 48.0                      # fp8 matmul bias row value (exact in e4m3)
DL8 = 16.0                      # fp8 prob down-scale 2^(-DL8/8) (softmax-inv.)
C16 = 7.4                       # Schraudolph tuning constants (mean-zeroing)
C8 = 0.45
B16F = 127.0 * 128.0 + 0.5 - C16          # DVE add for int16 path
D8F = 7.0 * 8.0 + 0.5 - C8 - BR8 - DL8    # DVE add for int8 path
TMV = -16000.0                  # causal mask add (upper triangle)

_build_cache = {}


def split_multiwaits(nc):
    """Split instructions carrying >1 sem-wait into single-wait NoOp + inst."""
    import concourse.mybir as mybir

    ctr = 0
    for fn in nc.m.functions:
        for bb in fn.blocks:
            out, changed = [], False
            for inst in list(bb.instructions):
                si = inst.sync_info
                waits = list(si.on_wait) if (si is not None and si.on_wait) else []
                if len(waits) > 1:
                    for w in waits[:-1]:
                        ctr += 1
                        out.append(
                            mybir.InstNoOp(
                                name=f"splitwait-{ctr}",
                                engine=inst.engine,
                                sync_info=mybir.SyncInfo(on_wait=[w], on_update=[]),
                            )
                        )
                    si.on_wait = waits[-1:]
                    changed = True
                out.append(inst)
            if changed:
                bb.instructions = out
    return ctr


# ---------------------------------------------------------------- scheduling

# cost-model constants (ns) for the greedy two-engine exp balancer
_ACT_RATE, _ACT_FIX = 1.0 / 1.2, 185.0
_DVE_RATE, _DVE_FIX = 1.0 / 0.96, 125.0
_EVAC_N = 2 * VF * 2            # two heads' [128, 4, 65] copies, in cols


def _steps(cfg):
    pg = cfg.get("pair_group", 1)
    ngroups = PAIRS // pg
    order = cfg.get("j_order",
                    [[0, 1, 2, 3]] * (ngroups - 1) + [[1, 2, 3, 0]])
    return [(tuple(range(g * pg, (g + 1) * pg)), j)
            for g in range(ngroups) for j in order[g]]


def _exp_schedule(cfg):
    """Greedy ACT/DVE assignment for exp units and acc-evac copies.

    Returns ({(pair, j, t): eng}, {(pair, j, hi): eng}, clocks).
    Each exp unit covers both heads of one k-tile: n = 2*(SQ_BLK - o) cols.
    """
    bias = cfg.get("exp_bias", {})
    clocks = {"act": 0.0, "dve": 0.0}

    def cost(e, n):
        return (n * _ACT_RATE + _ACT_FIX if e == "act"
                else n * _DVE_RATE + _DVE_FIX)

    sched, evac = {}, {}
    for pairs, j in _steps(cfg):
        n_t = 4 * j + 4
        for t in range(n_t):
            o = 128 * (t - 4 * j) if t >= 4 * j else 0
            n = 2 * (SQ_BLK - o)
            for pair in pairs:
                force = cfg.get("force_act_j0") and j == 0
                picks = {}
                for e in ("act", "dve"):
                    picks[e] = clocks[e] + cost(e, n) + bias.get(e, 0.0)
                e = "act" if force else min(picks, key=picks.get)
                sched[(pair, j, t)] = e
                clocks[e] = picks[e] - bias.get(e, 0.0)
        # acc evacuation copies of the PREVIOUS step trail this step's exp;
        # approximate by charging them here.
        for pair in pairs:
            for hi in (0, 1):
                n = 4 * VF
                picks = {e: clocks[e] + cost(e, n) for e in ("act", "dve")}
                e = min(picks, key=picks.get)
                evac[(pair, j, hi)] = e
                clocks[e] = picks[e]
    return sched, evac, clocks


# ---------------------------------------------------------------- build

def _build(cfg=None):
    from contextlib import ExitStack

    import concourse.bass as bass
    import concourse.tile as tile
    from concourse import mybir

    f32 = mybir.dt.float32
    bf16 = mybir.dt.bfloat16
    f8 = mybir.dt.float8e4
    i16 = mybir.dt.int16
    i8 = mybir.dt.int8
    Exp = mybir.ActivationFunctionType.Exp
    Add = mybir.AluOpType.add
    Max = mybir.AluOpType.max
    DR = mybir.MatmulPerfMode.DoubleRow

    cfg = {**{"ps_bufs": 2, "psd_bufs": 1,
              "pp_bufs": 24, "pp8_bufs": 24, "qk_bufs": 2,
              "o_bufs": 16, "ov_bufs": 1, "exp_bias": {},
              "fp8_from": FP8_FROM, "pv_first": False,
              "ov_shared": False, "force_act_j0": False,
              "pv_after_tiles": 5},
           **(cfg or {})}
    fp8_from = cfg["fp8_from"]       # first block index computed in fp8+DR
    bq = fp8_from * SQ_BLK           # bf16 q columns (s < bq), bf16 k tiles
    bkt = 4 * fp8_from               # number of bf16 k-tiles / vA tiles
    TP = SKT // 2                    # tile-pairs = 8

    sched, evac_sched, _clocks = _exp_schedule(cfg)

    nc = bass.Bass(num_devices=NCORES)
    qT = nc.dram_tensor("qT", [PAIRS, P, bq], bf16, kind="ExternalInput")
    kT = nc.dram_tensor("kT", [PAIRS, P, bkt * P], bf16, kind="ExternalInput")
    vA = nc.dram_tensor("vA", [PAIRS, P, 2 * bkt * VF], bf16,
                        kind="ExternalInput")
    # 33 contraction rows per DR slot: h 0-31 plus a bias row (Q=1, K=BR8,
    # slot 1 zeroed) so PSUM holds Z8 = A8*S + BR8 directly.
    q8 = nc.dram_tensor("q8", [PAIRS, 66, 2 * (SQ - bq)], f8,
                        kind="ExternalInput")
    k8 = nc.dram_tensor("k8", [PAIRS, 66, 2 * SK], f8, kind="ExternalInput")
    v8 = nc.dram_tensor("v8", [PAIRS, P, 2 * TP * 2 * VF], f8,
                        kind="ExternalInput")
    tmc = nc.dram_tensor("tmc", [P, 2 * P], bf16, kind="ExternalInput")
    out = nc.dram_tensor("out", [PAIRS, NBLK, P, 4 * 2 * VF], bf16,
                         kind="ExternalOutput")

    with tile.TileContext(nc) as tc, ExitStack() as ctx:
        const = ctx.enter_context(tc.tile_pool(name="const", bufs=1))
        qk_pool = ctx.enter_context(tc.tile_pool(name="qk", bufs=cfg["qk_bufs"]))
        p_pool = ctx.enter_context(tc.tile_pool(name="p", bufs=cfg["pp_bufs"]))
        p8_pool = ctx.enter_context(
            tc.tile_pool(name="p8", bufs=cfg["pp8_bufs"]))
        o_pool = ctx.enter_context(tc.tile_pool(name="o", bufs=cfg["o_bufs"]))
        ps_qk = ctx.enter_context(
            tc.tile_pool(name="psqk", bufs=cfg["ps_bufs"], space="PSUM"))
        ps_dve = ctx.enter_context(
            tc.tile_pool(name="psdve", bufs=cfg["psd_bufs"], space="PSUM"))
        ps_ov = ctx.enter_context(
            tc.tile_pool(name="psov", bufs=cfg["ov_bufs"], space="PSUM"))

        tmid_sb = const.tile([P, 2, P], bf16)
        nc.sync.dma_start(tmid_sb, tmc[:].rearrange("p (i f) -> p i f", i=2))
        tm_sb = tmid_sb[:, 0, :]
        id_sb = tmid_sb[:, 1, :]
        bias8_sb = const.tile([P, 1], f32)
        nc.vector.memset(bias8_sb, -(BR8 + DL8) / (8.0 * A8))

        def load_pair(pair, split_first=False):
            # split DMA dispatch across the SP and ACT sequencers so the fill
            # isn't serialized on one queue; each TAG keeps a fixed queue so
            # same-slot rewrites stay queue-ordered.
            qT_sb = qk_pool.tile([P, bq], bf16, tag="qT")
            kT_sb = qk_pool.tile([P, bkt * P], bf16, tag="kT")
            vA_sb = qk_pool.tile([P, 2, bkt, VF], bf16, tag="vA")
            q8_sb = qk_pool.tile([97, 2, SQ - bq], f8, tag="q8")
            k8_sb = qk_pool.tile([97, 2, SK], f8, tag="k8")
            v8_sb = qk_pool.tile([P, 2, TP, 2, VF], f8, tag="v8")
            if split_first:
                cut = SQ_BLK
                nc.scalar.dma_start(kT_sb[:, :cut], kT[pair, :, :cut])
                nc.sync.dma_start(qT_sb[:, :cut], qT[pair, :, :cut])
                nc.sync.dma_start(kT_sb[:, cut:], kT[pair, :, cut:])
                nc.sync.dma_start(qT_sb[:, cut:], qT[pair, :, cut:])
            else:
                nc.sync.dma_start(qT_sb, qT[pair])
                nc.sync.dma_start(kT_sb, kT[pair])
            nc.sync.dma_start(
                vA_sb, vA[pair].rearrange("p (h t f) -> p h t f", h=2, f=VF))
            nc.sync.dma_start(
                q8_sb[0:33], q8[pair, 0:33].rearrange("p (i s) -> p i s", i=2))
            nc.sync.dma_start(
                q8_sb[64:97],
                q8[pair, 33:66].rearrange("p (i s) -> p i s", i=2))
            nc.sync.dma_start(
                k8_sb[0:33], k8[pair, 0:33].rearrange("p (i s) -> p i s", i=2))
            nc.sync.dma_start(
                k8_sb[64:97],
                k8[pair, 33:66].rearrange("p (i s) -> p i s", i=2))
            nc.sync.dma_start(
                v8_sb, v8[pair].rearrange("p (h t i f) -> p h t i f",
                                          h=2, i=2, f=VF))
            return qT_sb, kT_sb, vA_sb, q8_sb, k8_sb, v8_sb

        steps = _steps(cfg)
        pg = cfg.get("pair_group", 1)
        tiles_by_pair = {}
        for pr in steps[0][0]:
            tiles_by_pair[pr] = load_pair(pr, split_first=(pr == steps[0][0][0]))
        pending = None  # (pairs, j, pps) awaiting PV emission

        def emit_qk_exp(pairs, j, t_range=None, pps=None, pp8s=None):
            use8 = j >= fp8_from
            s0 = j * SQ_BLK
            pps = {pair: [] for pair in pairs} if pps is None else pps
            pp8s = {} if pp8s is None else pp8s
            for t in (t_range if t_range is not None
                      else range(4 * j + 4)):
                diag = t >= 4 * j
                o = 128 * (t - 4 * j) if diag else 0
                for pair in pairs:
                    qT_sb, kT_sb, _, q8_sb, k8_sb, _ = tiles_by_pair[pair]
                    eng = sched[(pair, j, t)]
                    pool_ps = ps_dve if eng == "dve" else ps_qk
                    ps = pool_ps.tile([P, 2, SQ_BLK], f32,
                                      tag="psD" if eng == "dve" else "psA")
                    k_sl = slice(t * P, (t + 1) * P)
                    for hi in (0, 1):
                        dst_ps = ps[:, hi, o:SQ_BLK]
                        tri_ps = ps[:, hi, o:o + P]
                        if use8:
                            nc.tensor.matmul(
                                dst_ps,
                                lhsT=k8_sb[64 * hi:64 * hi + 33, :, k_sl],
                                rhs=q8_sb[64 * hi:64 * hi + 33, :,
                                          s0 - bq + o:s0 - bq + SQ_BLK],
                                start=True, stop=not diag, perf_mode=DR,
                            )
                        else:
                            nc.tensor.matmul(
                                dst_ps,
                                lhsT=kT_sb[64 * hi:64 * hi + 64, k_sl],
                                rhs=qT_sb[64 * hi:64 * hi + 64,
                                          s0 + o:s0 + SQ_BLK],
                                start=True, stop=not diag,
                            )
                        if diag:
                            nc.tensor.matmul(
                                tri_ps,
                                lhsT=tm_sb, rhs=id_sb,
                                start=False, stop=True,
                            )
                    if use8:
                        if t % 2 == 0:
                            pp8s[pair] = p8_pool.tile(
                                [P, 2, 2, SQ_BLK], f8, tag="pp8", name="pp8")
                        ppt = pp8s[pair]
                        pps[pair].append((t, o, ppt))
                        dst = ppt[:, t % 2, :, o:]
                    else:
                        ppt = p_pool.tile([P, 2, SQ_BLK], bf16, tag="pp",
                                          name="pp")
                        pps[pair].append((t, o, ppt))
                        dst = ppt[:, :, o:]
                    src = ps[:, :, o:]
                    if eng == "act":
                        if use8:
                            nc.scalar.activation(
                                dst, src, Exp, scale=1.0 / (8.0 * A8),
                                bias=bias8_sb[:, 0:1])
                        else:
                            nc.scalar.activation(
                                dst, src, Exp, scale=1.0 / (8.0 * A16))
                    else:
                        if use8:
                            nc.vector.tensor_scalar(
                                dst.bitcast(i8), src, D8F, 0.0, Add, Max)
                        else:
                            nc.vector.tensor_scalar(
                                dst.bitcast(i16), src, B16F, 0.0, Add, Max)
            return pps

        def emit_pv_one(pair, pi, j, pps):
            _, _, vA_sb, _, _, v8_sb = tiles_by_pair[pair]
            use8 = j >= fp8_from
            if cfg["ov_shared"]:
                accs = [ps_ov.tile([P, 4, P], f32, tag="ov", name=f"acc{hi}")
                        for hi in (0, 1)]
            else:
                accs = [ps_ov.tile([P, 4, P], f32, tag=f"o{pi}{hi}",
                                   name=f"acc{hi}") for hi in (0, 1)]
            # build op list: (c, hi, lhsT, rhs, perf_mode)
            ops = []
            if use8:
                n_tp = (4 * j + 4) // 2
                for tp in range(n_tp):
                    pp8 = pps[2 * tp][2]
                    d0 = 2 * tp - 4 * j          # diag offset of slot-0 tile
                    d1 = d0 + 1
                    for hi in (0, 1):
                        if d0 >= 0:
                            ops.append((d0, hi,
                                        pp8[:, 0, hi, d0 * P:(d0 + 1) * P],
                                        v8_sb[:, hi, tp, 0, :], None))
                    for c in range(max(0, d1), 4):
                        for hi in (0, 1):
                            ops.append((c, hi,
                                        pp8[:, :, hi, c * P:(c + 1) * P],
                                        v8_sb[:, hi, tp, :, :], DR))
            else:
                for ti, entry in enumerate(pps):
                    t, o, pp = entry[0], entry[1], entry[2]
                    d = o // P
                    for c in range(d, 4):
                        for hi in (0, 1):
                            ops.append((c, hi,
                                        pp[:, hi, c * P:(c + 1) * P],
                                        vA_sb[:, hi, t, :], None))
            seen = {0: False, 1: False}
            last_i = {0: None, 1: None}
            for i, (c, hi, _, _, _) in enumerate(ops):
                last_i[hi] = i
            for i, (c, hi, lhsT, rhs, pm) in enumerate(ops):
                nc.tensor.matmul(
                    accs[hi][:, c, 0:VF], lhsT=lhsT, rhs=rhs,
                    start=not seen[hi], stop=(i == last_i[hi]),
                    perf_mode=pm,
                )
                seen[hi] = True
            out_sb = o_pool.tile([P, 4, 2, VF], bf16, tag="osb")
            for hi in (0, 1):
                if evac_sched[(pair, j, hi)] == "act":
                    nc.scalar.copy(out_sb[:, :, hi, :], accs[hi][:, :, 0:VF])
                else:
                    nc.vector.tensor_copy(out_sb[:, :, hi, :],
                                          accs[hi][:, :, 0:VF])
            nc.sync.dma_start(
                out[pair, j].rearrange("p (c h f) -> p c h f", c=4, h=2),
                out_sb)

        def emit_pv(pairs, j, pps):
            for pi, pair in enumerate(pairs):
                emit_pv_one(pair, pi, j, pps[pair])

        for i, (pairs, j) in enumerate(steps):
            if i % NBLK == 1 and pairs[-1] + 1 < PAIRS:
                for pr in range(pairs[-1] + 1, pairs[-1] + 1 + pg):
                    tiles_by_pair[pr] = load_pair(pr)
            pv_after = cfg.get("pv_after_tiles")
            if cfg["pv_first"]:
                if pending is not None:
                    emit_pv(*pending)
                pps = emit_qk_exp(pairs, j)
            elif pv_after is not None:
                # emit PV(prev) after the first few QK tiles: PE interleaves
                # PV work while the exp ring fills, and accs drain earlier
                n_t = 4 * j + 4
                cut = min(pv_after, n_t)
                pps, pp8s = {pair: [] for pair in pairs}, {}
                emit_qk_exp(pairs, j, range(0, cut), pps, pp8s)
                if pending is not None:
                    emit_pv(*pending)
                emit_qk_exp(pairs, j, range(cut, n_t), pps, pp8s)
            else:
                pps = emit_qk_exp(pairs, j)
                if pending is not None:
                    emit_pv(*pending)
            pending = (pairs, j, pps)
        emit_pv(*pending)

    split_multiwaits(nc)
    return nc


# ---------------------------------------------------------------- host side

def _prepare(query, key, value, attention_mask):
    import ml_dtypes

    bf = ml_dtypes.bfloat16
    f8 = ml_dtypes.float8_e4m3fn
    query = np.asarray(query, dtype=np.float32)
    key = np.asarray(key, dtype=np.float32)
    value = np.asarray(value, dtype=np.float32)
    mask = np.asarray(attention_mask).astype(bool)[:, 0]   # [B, SQ, SK]

    causal = ~np.tril(np.ones((SQ, SK), dtype=bool))
    assert (mask == causal[None]).all(), "kernel specialized to causal mask"

    cache_key = "v3"
    if cache_key not in _build_cache:
        _build_cache[cache_key] = _build()
    nc = _build_cache[cache_key]

    bq = FP8_FROM * SQ_BLK
    bkt = 4 * FP8_FROM
    TP = SKT // 2

    tm = np.zeros((P, P), np.float32)
    tm[np.triu_indices(P, 1)] = TMV          # tm[s, k] = TMV if k > s
    tmid = np.concatenate(
        [tm.astype(bf), np.eye(P, dtype=bf)], axis=1)  # [P, 2*P]

    in_maps = []
    for c in range(NCORES):
        b = c // (NCORES // B)
        np_lo = (c % (NCORES // B)) * HPC
        q_c = query[:, b, np_lo:np_lo + HPC, :]          # [SQ, 8, 64]
        k_c = key[:, b, np_lo:np_lo + HPC, :]
        v_c = value[:, b, np_lo:np_lo + HPC, :]
        # bf16: [PAIRS, 128, cols]; head A h-dim on rows 0-63, head B on 64-127
        # exp-arg scale A16 folded into q.
        qT_np = np.ascontiguousarray(
            (q_c[:bq] * A16).transpose(1, 2, 0)).reshape(
            PAIRS, P, bq).astype(bf)
        kT_np = np.ascontiguousarray(
            k_c[:bkt * P].transpose(1, 2, 0)).reshape(
            PAIRS, P, bkt * P).astype(bf)
        vA_np = np.empty((PAIRS, 2, bkt, P, VF), np.float32)
        vA_np[:, :, :, :, :HN] = v_c[:bkt * P].transpose(1, 0, 2).reshape(
            PAIRS, 2, bkt, P, HN)
        vA_np[:, :, :, :, HN] = 1.0
        vA_np = np.ascontiguousarray(
            vA_np.transpose(0, 3, 1, 2, 4)).reshape(
            PAIRS, P, 2 * bkt * VF).astype(bf)
        # fp8 DR layouts: [PAIRS, 66, 2, cols]; per head 33 rows: slot-i row p
        # holds h = i*32 + p for p < 32, row 32 is the bias row (Q=1/K=BR8 in
        # slot 0, zero in slot 1). Head A rows 0-32, head B rows 33-65.
        def dr_pack(x_c, ncols, bias):
            # x_c: [ncols, 8, 64] -> [PAIRS, 66, 2, ncols]
            arr = np.zeros((PAIRS, 2, 33, 2, ncols), np.float32)
            src = x_c.reshape(ncols, PAIRS, 2, 2, 32).transpose(1, 2, 4, 3, 0)
            arr[:, :, :32] = src                       # h rows
            arr[:, :, 32, 0, :] = bias                 # bias row, slot 0
            return np.ascontiguousarray(arr.reshape(
                PAIRS, 66, 2 * ncols)).astype(f8)

        q8_np = dr_pack(q_c[bq:] * A8, SQ - bq, 1.0)
        k8_np = dr_pack(k_c, SK, BR8)
        # v8[pair][k_part, hi, tp, slot, f]
        v8_np = np.empty((PAIRS, 2, TP, 2, P, VF), np.float32)
        v8_np[:, :, :, :, :, :HN] = v_c.transpose(1, 0, 2).reshape(
            PAIRS, 2, TP, 2, P, HN)
        v8_np[:, :, :, :, :, HN] = 1.0
        v8_np = np.ascontiguousarray(
            v8_np.transpose(0, 4, 1, 2, 3, 5)).reshape(
            PAIRS, P, 2 * TP * 2 * VF).astype(f8)
        in_maps.append({"qT": qT_np, "kT": kT_np, "vA": vA_np,
                        "q8": q8_np, "k8": k8_np, "v8": v8_np,
                        "tmc": tmid})
    return nc, in_maps


def _assemble(results):
    full = np.empty((SQ, B, NP * HN), np.float32)
    for c in range(NCORES):
        b = c // (NCORES // B)
        np_lo = (c % (NCORES // B)) * HPC
        o = results[c]["out"].astype(np.float32).reshape(
            PAIRS, NBLK, P, 4, 2, VF)
        # s = j*512 + cc*128 + p ; head = 2*pair + h
        o = o.transpose(1, 3, 2, 0, 4, 5).reshape(SQ, HPC, VF)
        ctx = o[:, :, :HN] / o[:, :, HN:HN + 1]
        full[:, b, np_lo * HN:(np_lo + HPC) * HN] = ctx.reshape(SQ, HPC * HN)
    return full


def _ensure_device_backend():
    from concourse._compat import axon_active

    if not axon_active():
        return
    import jax

    try:
        if len(jax.devices()) >= NCORES and jax.devices()[0].platform != "cpu":
            return
    except Exception:
        pass
    try:
        import jax.extend.backend as jeb

        jax.config.update("jax_platform_name", "")
        jeb.clear_backends()
        jax.devices()
    except Exception:
        pass


def kernel(query, key, value, attention_mask):
    from concourse.bass_utils import run_bass_kernel_spmd

    nc, in_maps = _prepare(query, key, value, attention_mask)
    _ensure_device_backend()
    res = run_bass_kernel_spmd(nc, in_maps, core_ids=list(range(NCORES)))
    return _assemble(res.results)


# revision 11
# speedup vs baseline: 1.2642x; 1.2642x over previous
"""Trainium2 Bass kernel for nn_DotProductAttention (SQ=SK=2048, B=2, NP=32, HN=64).

v3 design (8 NeuronCores, batch*heads sharded, 8 heads per core = 4 pairs):

  - S^T tiles [128 k, 2 heads, <=512 s] per (k-tile, sq-block) in PSUM.
    QK matmul: lhsT = K^T chunk (head A on partitions 0-63, head B on 64-127),
    rhs = Q^T bf16 for j<2; fp8 DoubleRow (33x2 rows incl. bias row) for j>=2.
  - The exp-arg scale is folded into Q on the host so PSUM holds
    Z = (2^m/ln2) * (S/8)  (m = mantissa bits of the prob dtype: 7 for bf16,
    3 for fp8e4m3; the fp8 path also adds +BR8 via its matmul bias row).
  - exp runs on TWO engines, statically load-balanced:
      ACT: true exp via activation(Exp, scale[, bias]) from PSUM.
      DVE: Schraudolph bit-trick: one tensor_scalar(add B, max 0) converts
           Z to int16/int8 (C-cast truncation; +0.5 folded into B) whose raw
           bits ARE bf16/fp8 probs (written through a bitcast view of the
           pp tile). max(.,0) clamps masked/underflowed scores to +0.0.
  - Causal mask: constant strictly-upper-triangular Tm (-16000) accumulated
    into diagonal-tile PSUM by one extra matmul; both exp paths then emit
    exactly 0. Diagonal tiles only compute s >= 128*t (live extent).
  - PV with pp stationary: out[128 s, 65] += pp_chunk^T @ V_aug over k-tiles;
    V_aug has a ones column so row 64 accumulates the softmax denominator.
    fp8 pp uses DoubleRow (two k-tiles per op, 0.5 cyc/col).
  - NO on-device normalize: the [128, 4, 65] accumulators are evacuated
    PSUM->SBUF as bf16 (one copy per head, on whichever exp engine the
    balancer picks) and DMA'd out; the host divides by the denominator.

The walrus build in this container only accepts ONE sync-wait per
instruction; split_multiwaits() rewrites the Tile-scheduled program.
"""

import math

import numpy as np

SQ, SK, B, NP, HN = 2048, 2048, 2, 32, 64
NCORES = 8
HPC = B * NP // NCORES          # heads per core = 8
PAIRS = HPC // 2                # 4
P = 128
SQ_BLK = 512
NBLK = SQ // SQ_BLK             # 4
SKT = SK // P                   # 16
VF = HN + 1                     # 65: V columns + ones column (denominator)
FP8_FROM = 2                    # first sq-block computed in fp8 + DoubleRow

# --- Schraudolph constants -------------------------------------------------
LN2 = math.log(2.0)
A16 = 128.0 / (8.0 * LN2)       # fold into bf16-path q: Z16 = A16 * S_raw
A8 = 8.0 / (8.0 * LN2)          # fold into fp8-path q:  Z8  = A8 * S_raw (+BR8)
BR8 = 48.0                      # fp8 matmul bias row value (exact in e4m3)
DL8 = 16.0                      # fp8 prob down-scale 2^(-DL8/8) (softmax-inv.)
C16 = 7.4                       # Schraudolph tuning constants (mean-zeroing)
C8 = 0.45
B16F = 127.0 * 128.0 + 0.5 - C16          # DVE add for int16 path
D8F = 7.0 * 8.0 + 0.5 - C8 - BR8 - DL8    # DVE add for int8 path
TMV = -16000.0                  # causal mask add (upper triangle)

_build_cache = {}


def split_multiwaits(nc):
    """Split instructions carrying >1 sem-wait into single-wait NoOp + inst."""
    import concourse.mybir as mybir

    ctr = 0
    for fn in nc.m.functions:
        for bb in fn.blocks:
            out, changed = [], False
            for inst in list(bb.instructions):
                si = inst.sync_info
                waits = list(si.on_wait) if (si is not None and si.on_wait) else []
                if len(waits) > 1:
                    for w in waits[:-1]:
                        ctr += 1
                        out.append(
                            mybir.InstNoOp(
                                name=f"splitwait-{ctr}",
                                engine=inst.engine,
                                sync_info=mybir.SyncInfo(on_wait=[w], on_update=[]),
                            )
                        )
                    si.on_wait = waits[-1:]
                    changed = True
                out.append(inst)
            if changed:
                bb.instructions = out
    return ctr


# ---------------------------------------------------------------- scheduling

# cost-model constants (ns) for the greedy two-engine exp balancer
_ACT_RATE, _ACT_FIX = 1.0 / 1.2, 185.0
_DVE_RATE, _DVE_FIX = 1.0 / 0.96, 125.0
_EVAC_N = 2 * VF * 2            # two heads' [128, 4, 65] copies, in cols


def _steps(cfg):
    pg = cfg.get("pair_group", 1)
    ngroups = PAIRS // pg
    order = cfg.get("j_order",
                    [[0, 1, 2, 3]] * (ngroups - 1) + [[1, 2, 3, 0]])
    return [(tuple(range(g * pg, (g + 1) * pg)), j)
            for g in range(ngroups) for j in order[g]]


def _exp_schedule(cfg):
    """Greedy ACT/DVE assignment for exp units and acc-evac copies.

    Returns ({(pair, j, t): eng}, {(pair, j, hi): eng}, clocks).
    Each exp unit covers both heads of one k-tile: n = 2*(SQ_BLK - o) cols.
    """
    bias = cfg.get("exp_bias", {})
    clocks = {"act": 0.0, "dve": 0.0}

    def cost(e, n):
        return (n * _ACT_RATE + _ACT_FIX if e == "act"
                else n * _DVE_RATE + _DVE_FIX)

    sched, evac = {}, {}
    for pairs, j in _steps(cfg):
        units = [(t, 2 * SQ_BLK) for t in range(4 * j)] + [
            ("X", 2 * SQ_BLK), ("Y", 2 * SQ_BLK), ("Z", SQ_BLK)]
        for t, n in units:
            for pair in pairs:
                force = cfg.get("force_act_j0") and j == 0
                picks = {}
                for e in ("act", "dve"):
                    picks[e] = clocks[e] + cost(e, n) + bias.get(e, 0.0)
                e = "act" if force else min(picks, key=picks.get)
                sched[(pair, j, t)] = e
                clocks[e] = picks[e] - bias.get(e, 0.0)
        # acc evacuation copies of the PREVIOUS step trail this step's exp;
        # approximate by charging them here.
        for pair in pairs:
            for hi in (0, 1):
                n = 4 * VF
                picks = {e: clocks[e] + cost(e, n) for e in ("act", "dve")}
                e = min(picks, key=picks.get)
                evac[(pair, j, hi)] = e
                clocks[e] = picks[e]
    return sched, evac, clocks


# ---------------------------------------------------------------- build

def _build(cfg=None):
    from contextlib import ExitStack

    import concourse.bass as bass
    import concourse.tile as tile
    from concourse import mybir

    f32 = mybir.dt.float32
    bf16 = mybir.dt.bfloat16
    f8 = mybir.dt.float8e4
    i16 = mybir.dt.int16
    i8 = mybir.dt.int8
    Exp = mybir.ActivationFunctionType.Exp
    Add = mybir.AluOpType.add
    Max = mybir.AluOpType.max
    DR = mybir.MatmulPerfMode.DoubleRow

    cfg = {**{"ps_bufs": 3, "psd_bufs": 0,
              "pp_bufs": 24, "pp8_bufs": 24, "qk_bufs": 2,
              "o_bufs": 16, "ov_bufs": 1, "exp_bias": {},
              "fp8_from": FP8_FROM, "pv_first": False,
              "ov_shared": False, "force_act_j0": False,
              "pv_after_tiles": 5},
           **(cfg or {})}
    fp8_from = cfg["fp8_from"]       # first block index computed in fp8+DR
    bq = fp8_from * SQ_BLK           # bf16 q columns (s < bq), bf16 k tiles
    bkt = 4 * fp8_from               # number of bf16 k-tiles / vA tiles
    TP = SKT // 2                    # tile-pairs = 8

    sched, evac_sched, _clocks = _exp_schedule(cfg)

    nc = bass.Bass(num_devices=NCORES)
    qT = nc.dram_tensor("qT", [PAIRS, P, bq], bf16, kind="ExternalInput")
    kT = nc.dram_tensor("kT", [PAIRS, P, bkt * P], bf16, kind="ExternalInput")
    vA = nc.dram_tensor("vA", [PAIRS, P, 2 * bkt * VF], bf16,
                        kind="ExternalInput")
    # 33 contraction rows per DR slot: h 0-31 plus a bias row (Q=1, K=BR8,
    # slot 1 zeroed) so PSUM holds Z8 = A8*S + BR8 directly.
    q8 = nc.dram_tensor("q8", [PAIRS, 66, 2 * (SQ - bq)], f8,
                        kind="ExternalInput")
    k8 = nc.dram_tensor("k8", [PAIRS, 66, 2 * SK], f8, kind="ExternalInput")
    v8 = nc.dram_tensor("v8", [PAIRS, P, 2 * TP * 2 * VF], f8,
                        kind="ExternalInput")
    tmc = nc.dram_tensor("tmc", [P, 2 * P], bf16, kind="ExternalInput")
    out = nc.dram_tensor("out", [PAIRS, NBLK, P, 4 * 2 * VF], bf16,
                         kind="ExternalOutput")

    with tile.TileContext(nc) as tc, ExitStack() as ctx:
        const = ctx.enter_context(tc.tile_pool(name="const", bufs=1))
        qk_pool = ctx.enter_context(tc.tile_pool(name="qk", bufs=cfg["qk_bufs"]))
        p_pool = ctx.enter_context(tc.tile_pool(name="p", bufs=cfg["pp_bufs"]))
        p8_pool = ctx.enter_context(
            tc.tile_pool(name="p8", bufs=cfg["pp8_bufs"]))
        o_pool = ctx.enter_context(tc.tile_pool(name="o", bufs=cfg["o_bufs"]))
        ps_qk = ctx.enter_context(
            tc.tile_pool(name="psqk", bufs=cfg["ps_bufs"], space="PSUM"))
        ps_dve = (ctx.enter_context(
            tc.tile_pool(name="psdve", bufs=cfg["psd_bufs"], space="PSUM"))
            if cfg["psd_bufs"] else ps_qk)
        ps_ov = ctx.enter_context(
            tc.tile_pool(name="psov", bufs=cfg["ov_bufs"], space="PSUM"))

        tmid_sb = const.tile([P, 2, P], bf16)
        tm_sb = tmid_sb[:, 0, :]
        id_sb = tmid_sb[:, 1, :]
        bias8_sb = const.tile([P, 1], f32)
        nc.vector.memset(bias8_sb, -(BR8 + DL8) / (8.0 * A8))

        def load_pair(pair, split_first=False):
            # split DMA dispatch across the SP and ACT sequencers so the fill
            # isn't serialized on one queue; each TAG keeps a fixed queue so
            # same-slot rewrites stay queue-ordered.
            qT_sb = qk_pool.tile([P, bq], bf16, tag="qT")
            kT_sb = qk_pool.tile([P, bkt * P], bf16, tag="kT")
            vA_sb = qk_pool.tile([P, 2, bkt, VF], bf16, tag="vA")
            q8_sb = qk_pool.tile([97, 2, SQ - bq], f8, tag="q8")
            k8_sb = qk_pool.tile([97, 2, SK], f8, tag="k8")
            v8_sb = qk_pool.tile([P, 2, TP, 2, VF], f8, tag="v8")
            if split_first:
                cut = SQ_BLK
                nc.sync.dma_start(kT_sb[:, :P], kT[pair, :, :P])
                nc.scalar.dma_start(qT_sb[:, :cut], qT[pair, :, :cut])
                nc.gpsimd.dma_start(
                    tmid_sb, tmc[:].rearrange("p (i f) -> p i f", i=2))
                nc.scalar.dma_start(kT_sb[:, P:cut], kT[pair, :, P:cut])
                nc.sync.dma_start(kT_sb[:, cut:], kT[pair, :, cut:])
                nc.sync.dma_start(qT_sb[:, cut:], qT[pair, :, cut:])
            else:
                nc.sync.dma_start(qT_sb, qT[pair])
                nc.scalar.dma_start(kT_sb, kT[pair])
            nc.gpsimd.dma_start(
                vA_sb, vA[pair].rearrange("p (h t f) -> p h t f", h=2, f=VF))
            nc.gpsimd.dma_start(
                q8_sb[0:33], q8[pair, 0:33].rearrange("p (i s) -> p i s", i=2))
            nc.gpsimd.dma_start(
                q8_sb[64:97],
                q8[pair, 33:66].rearrange("p (i s) -> p i s", i=2))
            nc.gpsimd.dma_start(
                k8_sb[0:33], k8[pair, 0:33].rearrange("p (i s) -> p i s", i=2))
            nc.gpsimd.dma_start(
                k8_sb[64:97],
                k8[pair, 33:66].rearrange("p (i s) -> p i s", i=2))
            nc.gpsimd.dma_start(
                v8_sb, v8[pair].rearrange("p (h t i f) -> p h t i f",
                                          h=2, i=2, f=VF))
            return qT_sb, kT_sb, vA_sb, q8_sb, k8_sb, v8_sb

        steps = _steps(cfg)
        pg = cfg.get("pair_group", 1)
        tiles_by_pair = {}
        for pr in steps[0][0]:
            tiles_by_pair[pr] = load_pair(pr, split_first=(pr == steps[0][0][0]))
        pending = None  # (pairs, j, pps) awaiting PV emission

        def emit_qk_exp(pairs, j, u_range=None, pps=None, pp8s=None,
                        pv_queue=None, pv_per_tile=0):
            """Emit QK matmuls + exp for a range of UNITS.

            Units: 0..4j-1 are full k-tiles; then "X" (diag t0, 512 cols),
            "Y" (diag t1 @0:384 + diag t3 @384:512), "Z" (diag t2 @0:256).
            pps[pair] entries: (t, o, ppt, base, slot) — the pp storage
            position of s-col s is base + (s - o); slot is the pp8 DR slot
            index for full fp8 tiles, else None.
            """
            use8 = j >= fp8_from
            s0 = j * SQ_BLK
            pps = {pair: [] for pair in pairs} if pps is None else pps
            pp8s = {} if pp8s is None else pp8s
            d0 = 4 * j                       # first diag tile index
            all_units = list(range(4 * j)) + ["X", "Y", "Z"]
            for u in (u_range if u_range is not None else all_units):
                if pv_queue:
                    for _ in range(min(pv_per_tile, len(pv_queue))):
                        pv_queue.pop(0)()
                # (tile, o, ps-col-base, ncols) sub-blocks of this unit
                if u == "X":
                    blocks = [(d0, 0, 0, SQ_BLK)]
                elif u == "Y":
                    blocks = [(d0 + 1, 128, 0, 384), (d0 + 3, 384, 384, 128)]
                elif u == "Z":
                    blocks = [(d0 + 2, 256, 0, 256)]
                else:
                    blocks = [(u, 0, 0, SQ_BLK)]
                ncols = sum(b[3] for b in blocks)
                for pair in pairs:
                    qT_sb, kT_sb, _, q8_sb, k8_sb, _ = tiles_by_pair[pair]
                    eng = sched[(pair, j, u)]
                    ps = ps_qk.tile([P, 2, SQ_BLK], f32, tag="ps")
                    for t, o, cb, n in blocks:
                        diag = t >= d0
                        k_sl = slice(t * P, (t + 1) * P)
                        for hi in (0, 1):
                            dst_ps = ps[:, hi, cb:cb + n]
                            tri_ps = ps[:, hi, cb:cb + P]
                            if use8:
                                nc.tensor.matmul(
                                    dst_ps,
                                    lhsT=k8_sb[64 * hi:64 * hi + 33, :, k_sl],
                                    rhs=q8_sb[64 * hi:64 * hi + 33, :,
                                              s0 - bq + o:s0 - bq + o + n],
                                    start=True, stop=not diag, perf_mode=DR,
                                )
                            else:
                                nc.tensor.matmul(
                                    dst_ps,
                                    lhsT=kT_sb[64 * hi:64 * hi + 64, k_sl],
                                    rhs=qT_sb[64 * hi:64 * hi + 64,
                                              s0 + o:s0 + o + n],
                                    start=True, stop=not diag,
                                )
                            if diag:
                                nc.tensor.matmul(
                                    tri_ps,
                                    lhsT=tm_sb, rhs=id_sb,
                                    start=False, stop=True,
                                )
                    # pp destination
                    if isinstance(u, int) and use8:
                        if u % 2 == 0:
                            pp8s[pair] = p8_pool.tile(
                                [P, 2, 2, SQ_BLK], f8, tag="pp8", name="pp8")
                        ppt = pp8s[pair]
                        pps[pair].append((u, 0, ppt, 0, u % 2))
                        dst = ppt[:, u % 2, :, :]
                    else:
                        dt_ = f8 if use8 else bf16
                        tag_ = "pp8d" if use8 else "pp"
                        pool_ = p8_pool if use8 else p_pool
                        ppt = pool_.tile([P, 2, SQ_BLK], dt_, tag=tag_,
                                         name=tag_)
                        for t, o, cb, n in blocks:
                            pps[pair].append((t, o, ppt, cb, None))
                        dst = ppt[:, :, 0:ncols] if ncols < SQ_BLK else ppt[:]
                    src_ps = ps[:, :, 0:ncols]
                    if u == "Y":            # non-contiguous: cols 0:512 full
                        src_ps, dst = ps[:], (ppt[:, :, :] if True else dst)
                    if eng == "act":
                        if use8:
                            nc.scalar.activation(
                                dst, src_ps, Exp, scale=1.0 / (8.0 * A8),
                                bias=bias8_sb[:, 0:1])
                        else:
                            nc.scalar.activation(
                                dst, src_ps, Exp, scale=1.0 / (8.0 * A16))
                    else:
                        if use8:
                            nc.vector.tensor_scalar(
                                dst.bitcast(i8), src_ps, D8F, 0.0, Add, Max)
                        else:
                            nc.vector.tensor_scalar(
                                dst.bitcast(i16), src_ps, B16F, 0.0, Add, Max)
            return pps

        def pv_closures(pair, pi, j, pps):
            _, _, vA_sb, _, _, v8_sb = tiles_by_pair[pair]
            use8 = j >= fp8_from
            if cfg["ov_shared"]:
                accs = [ps_ov.tile([P, 4, P], f32, tag="ov", name=f"acc{hi}")
                        for hi in (0, 1)]
            else:
                accs = [ps_ov.tile([P, 4, P], f32, tag=f"o{pi}{hi}",
                                   name=f"acc{hi}") for hi in (0, 1)]
            # build op list: (c, hi, lhsT, rhs, perf_mode)
            ops = []
            if use8:
                full = [e for e in pps if e[4] is not None]
                diag = [e for e in pps if e[4] is None]
                for fi in range(0, len(full), 2):
                    pp8 = full[fi][2]
                    tp = full[fi][0] // 2
                    for c in range(4):
                        for hi in (0, 1):
                            ops.append((c, hi,
                                        pp8[:, :, hi, c * P:(c + 1) * P],
                                        v8_sb[:, hi, tp, :, :], DR))
                for t, o, ppt, base, _ in diag:
                    tp, sl = t // 2, t % 2
                    for c in range(o // P, 4):
                        x = base + c * P - o
                        for hi in (0, 1):
                            ops.append((c, hi,
                                        ppt[:, hi, x:x + P],
                                        v8_sb[:, hi, tp, sl, :], None))
            else:
                for t, o, ppt, base, _ in pps:
                    for c in range(o // P, 4):
                        x = base + c * P - o
                        for hi in (0, 1):
                            ops.append((c, hi,
                                        ppt[:, hi, x:x + P],
                                        vA_sb[:, hi, t, :], None))
            seen = {0: False, 1: False}
            last_i = {0: None, 1: None}
            for i, (c, hi, _, _, _) in enumerate(ops):
                last_i[hi] = i
            closures = []
            for i, (c, hi, lhsT, rhs, pm) in enumerate(ops):
                def mk(i=i, c=c, hi=hi, lhsT=lhsT, rhs=rhs, pm=pm):
                    nc.tensor.matmul(
                        accs[hi][:, c, 0:VF], lhsT=lhsT, rhs=rhs,
                        start=not seen[hi], stop=(i == last_i[hi]),
                        perf_mode=pm,
                    )
                    seen[hi] = True
                closures.append(mk)

            def finish():
                out_sb = o_pool.tile([P, 4, 2, VF], bf16, tag="osb")
                for hi in (0, 1):
                    if evac_sched[(pair, j, hi)] == "act":
                        nc.scalar.copy(out_sb[:, :, hi, :],
                                       accs[hi][:, :, 0:VF])
                    else:
                        nc.vector.tensor_copy(out_sb[:, :, hi, :],
                                              accs[hi][:, :, 0:VF])
                nc.sync.dma_start(
                    out[pair, j].rearrange("p (c h f) -> p c h f", c=4, h=2),
                    out_sb)
            closures.append(finish)
            return closures

        def emit_pv(pairs, j, pps):
            for pi, pair in enumerate(pairs):
                for cl in pv_closures(pair, pi, j, pps[pair]):
                    cl()

        lead = cfg.get("pv_lead", 2)
        for i, (pairs, j) in enumerate(steps):
            if i % NBLK == 1 and pairs[-1] + 1 < PAIRS:
                for pr in range(pairs[-1] + 1, pairs[-1] + 1 + pg):
                    tiles_by_pair[pr] = load_pair(pr)
            all_units = list(range(4 * j)) + ["X", "Y", "Z"]
            n_u = len(all_units)
            pvq = []
            if pending is not None:
                ppairs, pj, ppps = pending
                for pi, pr in enumerate(ppairs):
                    pvq.extend(pv_closures(pr, pi, pj, ppps[pr]))
            # spread pending-PV ops across the QK units after a short lead
            n_gaps = max(1, n_u - lead)
            per_tile = -(-len(pvq) // n_gaps) if pvq else 0
            pps, pp8s = {pair: [] for pair in pairs}, {}
            emit_qk_exp(pairs, j, all_units[:lead], pps, pp8s)
            emit_qk_exp(pairs, j, all_units[lead:], pps, pp8s,
                        pv_queue=pvq, pv_per_tile=per_tile)
            while pvq:
                pvq.pop(0)()
            pending = (pairs, j, pps)
        emit_pv(*pending)

    split_multiwaits(nc)
    return nc


# ---------------------------------------------------------------- host side

def _prepare(query, key, value, attention_mask):
    import ml_dtypes

    bf = ml_dtypes.bfloat16
    f8 = ml_dtypes.float8_e4m3fn
    query = np.asarray(query, dtype=np.float32)
    key = np.asarray(key, dtype=np.float32)
    value = np.asarray(value, dtype=np.float32)
    mask = np.asarray(attention_mask).astype(bool)[:, 0]   # [B, SQ, SK]

    causal = ~np.tril(np.ones((SQ, SK), dtype=bool))
    assert (mask == causal[None]).all(), "kernel specialized to causal mask"

    cache_key = "v3"
    if cache_key not in _build_cache:
        _build_cache[cache_key] = _build()
    nc = _build_cache[cache_key]

    bq = FP8_FROM * SQ_BLK
    bkt = 4 * FP8_FROM
    TP = SKT // 2

    tm = np.zeros((P, P), np.float32)
    tm[np.triu_indices(P, 1)] = TMV          # tm[s, k] = TMV if k > s
    tmid = np.concatenate(
        [tm.astype(bf), np.eye(P, dtype=bf)], axis=1)  # [P, 2*P]

    in_maps = []
    for c in range(NCORES):
        b = c // (NCORES // B)
        np_lo = (c % (NCORES // B)) * HPC
        q_c = query[:, b, np_lo:np_lo + HPC, :]          # [SQ, 8, 64]
        k_c = key[:, b, np_lo:np_lo + HPC, :]
        v_c = value[:, b, np_lo:np_lo + HPC, :]
        # bf16: [PAIRS, 128, cols]; head A h-dim on rows 0-63, head B on 64-127
        # exp-arg scale A16 folded into q.
        qT_np = np.ascontiguousarray(
            (q_c[:bq] * A16).transpose(1, 2, 0)).reshape(
            PAIRS, P, bq).astype(bf)
        kT_np = np.ascontiguousarray(
            k_c[:bkt * P].transpose(1, 2, 0)).reshape(
            PAIRS, P, bkt * P).astype(bf)
        vA_np = np.empty((PAIRS, 2, bkt, P, VF), np.float32)
        vA_np[:, :, :, :, :HN] = v_c[:bkt * P].transpose(1, 0, 2).reshape(
            PAIRS, 2, bkt, P, HN)
        vA_np[:, :, :, :, HN] = 1.0
        vA_np = np.ascontiguousarray(
            vA_np.transpose(0, 3, 1, 2, 4)).reshape(
            PAIRS, P, 2 * bkt * VF).astype(bf)
        # fp8 DR layouts: [PAIRS, 66, 2, cols]; per head 33 rows: slot-i row p
        # holds h = i*32 + p for p < 32, row 32 is the bias row (Q=1/K=BR8 in
        # slot 0, zero in slot 1). Head A rows 0-32, head B rows 33-65.
        def dr_pack(x_c, ncols, bias):
            # x_c: [ncols, 8, 64] -> [PAIRS, 66, 2, ncols]
            arr = np.zeros((PAIRS, 2, 33, 2, ncols), np.float32)
            src = x_c.reshape(ncols, PAIRS, 2, 2, 32).transpose(1, 2, 4, 3, 0)
            arr[:, :, :32] = src                       # h rows
            arr[:, :, 32, 0, :] = bias                 # bias row, slot 0
            return np.ascontiguousarray(arr.reshape(
                PAIRS, 66, 2 * ncols)).astype(f8)

        q8_np = dr_pack(q_c[bq:] * A8, SQ - bq, 1.0)
        k8_np = dr_pack(k_c, SK, BR8)
        # v8[pair][k_part, hi, tp, slot, f]
        v8_np = np.empty((PAIRS, 2, TP, 2, P, VF), np.float32)
        v8_np[:, :, :, :, :, :HN] = v_c.transpose(1, 0, 2).reshape(
            PAIRS, 2, TP, 2, P, HN)
        v8_np[:, :, :, :, :, HN] = 1.0
        v8_np = np.ascontiguousarray(
            v8_np.transpose(0, 4, 1, 2, 3, 5)).reshape(
            PAIRS, P, 2 * TP * 2 * VF).astype(f8)
        in_maps.append({"qT": qT_np, "kT": kT_np, "vA": vA_np,
                        "q8": q8_np, "k8": k8_np, "v8": v8_np,
                        "tmc": tmid})
    return nc, in_maps


def _assemble(results):
    full = np.empty((SQ, B, NP * HN), np.float32)
    for c in range(NCORES):
        b = c // (NCORES // B)
        np_lo = (c % (NCORES // B)) * HPC
        o = results[c]["out"].astype(np.float32).reshape(
            PAIRS, NBLK, P, 4, 2, VF)
        # s = j*512 + cc*128 + p ; head = 2*pair + h
        o = o.transpose(1, 3, 2, 0, 4, 5).reshape(SQ, HPC, VF)
        ctx = o[:, :, :HN] / o[:, :, HN:HN + 1]
        full[:, b, np_lo * HN:(np_lo + HPC) * HN] = ctx.reshape(SQ, HPC * HN)
    return full


def _ensure_device_backend():
    from concourse._compat import axon_active

    if not axon_active():
        return
    import jax

    try:
        if len(jax.devices()) >= NCORES and jax.devices()[0].platform != "cpu":
            return
    except Exception:
        pass
    try:
        import jax.extend.backend as jeb

        jax.config.update("jax_platform_name", "")
        jeb.clear_backends()
        jax.devices()
    except Exception:
        pass


def kernel(query, key, value, attention_mask):
    from concourse.bass_utils import run_bass_kernel_spmd

    nc, in_maps = _prepare(query, key, value, attention_mask)
    _ensure_device_backend()
    res = run_bass_kernel_spmd(nc, in_maps, core_ids=list(range(NCORES)))
    return _assemble(res.results)


# revision 20
# speedup vs baseline: 1.3211x; 1.0450x over previous
"""Trainium2 Bass kernel for nn_DotProductAttention (SQ=SK=2048, B=2, NP=32, HN=64).

v3 design (8 NeuronCores, batch*heads sharded, 8 heads per core = 4 pairs):

  - S^T tiles [128 k, 2 heads, <=512 s] per (k-tile, sq-block) in PSUM.
    QK matmul: lhsT = K^T chunk (head A on partitions 0-63, head B on 64-127),
    rhs = Q^T bf16 for j<2; fp8 DoubleRow (33x2 rows incl. bias row) for j>=2.
  - The exp-arg scale is folded into Q on the host so PSUM holds
    Z = (2^m/ln2) * (S/8)  (m = mantissa bits of the prob dtype: 7 for bf16,
    3 for fp8e4m3; the fp8 path also adds +BR8 via its matmul bias row).
  - exp runs on TWO engines, statically load-balanced:
      ACT: true exp via activation(Exp, scale[, bias]) from PSUM.
      DVE: Schraudolph bit-trick: one tensor_scalar(add B, max 0) converts
           Z to int16/int8 (C-cast truncation; +0.5 folded into B) whose raw
           bits ARE bf16/fp8 probs (written through a bitcast view of the
           pp tile). max(.,0) clamps masked/underflowed scores to +0.0.
  - Causal mask: constant strictly-upper-triangular Tm (-16000) accumulated
    into diagonal-tile PSUM by one extra matmul; both exp paths then emit
    exactly 0. Diagonal tiles only compute s >= 128*t (live extent).
  - PV with pp stationary: out[128 s, 65] += pp_chunk^T @ V_aug over k-tiles;
    V_aug has a ones column so row 64 accumulates the softmax denominator.
    fp8 pp uses DoubleRow (two k-tiles per op, 0.5 cyc/col).
  - NO on-device normalize: the [128, 4, 65] accumulators are evacuated
    PSUM->SBUF as bf16 (one copy per head, on whichever exp engine the
    balancer picks) and DMA'd out; the host divides by the denominator.

The walrus build in this container only accepts ONE sync-wait per
instruction; split_multiwaits() rewrites the Tile-scheduled program.
"""

import math

import numpy as np

SQ, SK, B, NP, HN = 2048, 2048, 2, 32, 64
NCORES = 8
HPC = B * NP // NCORES          # heads per core = 8
PAIRS = HPC // 2                # 4
P = 128
SQ_BLK = 512
NBLK = SQ // SQ_BLK             # 4
SKT = SK // P                   # 16
VF = HN + 1                     # 65: V columns + ones column (denominator)
FP8_FROM = 2                    # first sq-block computed in fp8 + DoubleRow

# --- Schraudolph constants -------------------------------------------------
LN2 = math.log(2.0)
A16 = 128.0 / (8.0 * LN2)       # fold into bf16-path q: Z16 = A16 * S_raw
A8 = 8.0 / (8.0 * LN2)          # fold into fp8-path q:  Z8  = A8 * S_raw (+BR8)
BR8 = 48.0                      # fp8 matmul bias row value (exact in e4m3)
DL8 = 16.0                      # fp8 prob down-scale 2^(-DL8/8) (softmax-inv.)
C16 = 7.4                       # Schraudolph tuning constants (mean-zeroing)
C8 = 0.45
B16F = 127.0 * 128.0 + 0.5 - C16          # DVE add for int16 path
D8F = 7.0 * 8.0 + 0.5 - C8 - BR8 - DL8    # DVE add for int8 path
TMV = -16000.0                  # causal mask add (upper triangle)

_build_cache = {}


def split_multiwaits(nc):
    """Split instructions carrying >1 sem-wait into single-wait NoOp + inst."""
    import concourse.mybir as mybir

    ctr = 0
    for fn in nc.m.functions:
        for bb in fn.blocks:
            out, changed = [], False
            for inst in list(bb.instructions):
                si = inst.sync_info
                waits = list(si.on_wait) if (si is not None and si.on_wait) else []
                if len(waits) > 1:
                    for w in waits[:-1]:
                        ctr += 1
                        out.append(
                            mybir.InstNoOp(
                                name=f"splitwait-{ctr}",
                                engine=inst.engine,
                                sync_info=mybir.SyncInfo(on_wait=[w], on_update=[]),
                            )
                        )
                    si.on_wait = waits[-1:]
                    changed = True
                out.append(inst)
            if changed:
                bb.instructions = out
    return ctr


# ---------------------------------------------------------------- scheduling

# cost-model constants (ns) for the greedy two-engine exp balancer
_ACT_RATE, _ACT_FIX = 1.0 / 1.2, 185.0
_DVE_RATE, _DVE_FIX = 1.0 / 0.96, 125.0
_EVAC_N = 2 * VF * 2            # two heads' [128, 4, 65] copies, in cols


def _steps(cfg):
    pg = cfg.get("pair_group", 1)
    ngroups = PAIRS // pg
    order = cfg.get("j_order",
                    [[0, 1, 2, 3]] * (ngroups - 1) + [[1, 2, 3, 0]])
    return [(tuple(range(g * pg, (g + 1) * pg)), j)
            for g in range(ngroups) for j in order[g]]


def _exp_schedule(cfg):
    """Greedy ACT/DVE assignment for exp units and acc-evac copies.

    Returns ({(pair, j, t): eng}, {(pair, j, hi): eng}, clocks).
    Each exp unit covers both heads of one k-tile: n = 2*(SQ_BLK - o) cols.
    """
    bias = cfg.get("exp_bias", {})
    clocks = {"act": 0.0, "dve": 0.0}

    def cost(e, n):
        return (n * _ACT_RATE + _ACT_FIX if e == "act"
                else n * _DVE_RATE + _DVE_FIX)

    sched, evac = {}, {}
    pat = cfg.get("alt_pattern")      # e.g. "ADADADA" cycled over units
    k = 0
    for pairs, j in _steps(cfg):
        units = [(t, 2 * SQ_BLK) for t in range(4 * j)] + [
            ("X", 2 * SQ_BLK), ("Y", 2 * SQ_BLK), ("Z", SQ_BLK)]
        for t, n in units:
            for pair in pairs:
                force = cfg.get("force_act_j0") and j == 0
                picks = {}
                for e in ("act", "dve"):
                    picks[e] = clocks[e] + cost(e, n) + bias.get(e, 0.0)
                if pat:
                    e = "act" if pat[k % len(pat)] == "A" else "dve"
                    k += 1
                else:
                    e = "act" if force else min(picks, key=picks.get)
                sched[(pair, j, t)] = e
                clocks[e] = picks[e] - bias.get(e, 0.0)
        # acc evacuation copies of the PREVIOUS step trail this step's exp;
        # approximate by charging them here.
        for pair in pairs:
            n = 2 * 4 * VF
            picks = {e: clocks[e] + cost(e, n) for e in ("act", "dve")}
            e = min(picks, key=picks.get)
            evac[(pair, j, 0)] = e
            clocks[e] = picks[e]
    return sched, evac, clocks


# ---------------------------------------------------------------- build

def _build(cfg=None):
    from contextlib import ExitStack

    import concourse.bass as bass
    import concourse.tile as tile
    from concourse import mybir

    f32 = mybir.dt.float32
    bf16 = mybir.dt.bfloat16
    f8 = mybir.dt.float8e4
    i16 = mybir.dt.int16
    i8 = mybir.dt.int8
    Exp = mybir.ActivationFunctionType.Exp
    Add = mybir.AluOpType.add
    Max = mybir.AluOpType.max
    DR = mybir.MatmulPerfMode.DoubleRow

    cfg = {**{"ps_bufs": 3, "psd_bufs": 0,
              "pp_bufs": 32, "pp8_bufs": 32, "qk_bufs": 2,
              "o_bufs": 16, "ov_bufs": 1, "exp_bias": {},
              "fp8_from": FP8_FROM, "pv_first": False,
              "ov_shared": False, "force_act_j0": False,
              "alt_pattern": "DA", "pv_slack": 6,
              "pv_after_tiles": 5},
           **(cfg or {})}
    fp8_from = cfg["fp8_from"]       # first block index computed in fp8+DR
    bq = fp8_from * SQ_BLK           # bf16 q columns (s < bq), bf16 k tiles
    bkt = 4 * fp8_from               # number of bf16 k-tiles / vA tiles
    TP = SKT // 2                    # tile-pairs = 8

    sched, evac_sched, _clocks = _exp_schedule(cfg)

    nc = bass.Bass(num_devices=NCORES)
    if fp8_from > 0:
        qT = nc.dram_tensor("qT", [PAIRS, P, bq], bf16, kind="ExternalInput")
        kT = nc.dram_tensor("kT", [PAIRS, P, bkt * P], bf16,
                            kind="ExternalInput")
        vA = nc.dram_tensor("vA", [PAIRS, P, 2 * bkt * VF], bf16,
                            kind="ExternalInput")
    # 33 contraction rows per DR slot: h 0-31 plus a bias row (Q=1, K=BR8,
    # slot 1 zeroed) so PSUM holds Z8 = A8*S + BR8 directly.
    q8 = nc.dram_tensor("q8", [PAIRS, 66, 2 * (SQ - bq)], f8,
                        kind="ExternalInput")
    k8 = nc.dram_tensor("k8", [PAIRS, 66, 2 * SK], f8, kind="ExternalInput")
    v8 = nc.dram_tensor("v8", [PAIRS, P, 2 * TP * 2 * VF], f8,
                        kind="ExternalInput")
    tmc = nc.dram_tensor("tmc", [P, 2 * P], bf16, kind="ExternalInput")
    out = nc.dram_tensor("out", [PAIRS, NBLK, P, 4 * 2 * VF], bf16,
                         kind="ExternalOutput")

    with tile.TileContext(nc) as tc, ExitStack() as ctx:
        const = ctx.enter_context(tc.tile_pool(name="const", bufs=1))
        qk_pool = ctx.enter_context(tc.tile_pool(name="qk", bufs=cfg["qk_bufs"]))
        p_pool = ctx.enter_context(tc.tile_pool(name="p", bufs=cfg["pp_bufs"]))
        p8_pool = ctx.enter_context(
            tc.tile_pool(name="p8", bufs=cfg["pp8_bufs"]))
        o_pool = ctx.enter_context(tc.tile_pool(name="o", bufs=cfg["o_bufs"]))
        ps_qk = ctx.enter_context(
            tc.tile_pool(name="psqk", bufs=cfg["ps_bufs"], space="PSUM"))
        ps_dve = (ctx.enter_context(
            tc.tile_pool(name="psdve", bufs=cfg["psd_bufs"], space="PSUM"))
            if cfg["psd_bufs"] else ps_qk)
        ps_ov = ctx.enter_context(
            tc.tile_pool(name="psov", bufs=cfg["ov_bufs"], space="PSUM"))

        tmid_sb = const.tile([P, 2, P], bf16)
        tm_sb = tmid_sb[:, 0, :]
        id_sb = tmid_sb[:, 1, :]
        bias8_sb = const.tile([P, 1], f32)
        nc.vector.memset(bias8_sb, -(BR8 + DL8) / (8.0 * A8))

        def load_pair(pair, split_first=False):
            # split DMA dispatch across the SP and ACT sequencers so the fill
            # isn't serialized on one queue; each TAG keeps a fixed queue so
            # same-slot rewrites stay queue-ordered.
            if fp8_from > 0:
                qT_sb = qk_pool.tile([P, bq], bf16, tag="qT")
                kT_sb = qk_pool.tile([P, bkt * P], bf16, tag="kT")
                vA_sb = qk_pool.tile([P, 2, bkt, VF], bf16, tag="vA")
            else:
                qT_sb = kT_sb = vA_sb = None
            q8_sb = qk_pool.tile([97, 2, SQ - bq], f8, tag="q8")
            k8_sb = qk_pool.tile([97, 2, SK], f8, tag="k8")
            v8_sb = qk_pool.tile([P, 2, TP, 2, VF], f8, tag="v8")
            if fp8_from == 0:
                if split_first:
                    nc.sync.dma_start(
                        k8_sb[0:33],
                        k8[pair, 0:33].rearrange("p (i s) -> p i s", i=2))
                    nc.scalar.dma_start(
                        q8_sb[0:33],
                        q8[pair, 0:33].rearrange("p (i s) -> p i s", i=2))
                    nc.gpsimd.dma_start(
                        tmid_sb, tmc[:].rearrange("p (i f) -> p i f", i=2))
                    nc.sync.dma_start(
                        k8_sb[64:97],
                        k8[pair, 33:66].rearrange("p (i s) -> p i s", i=2))
                    nc.scalar.dma_start(
                        q8_sb[64:97],
                        q8[pair, 33:66].rearrange("p (i s) -> p i s", i=2))
                else:
                    nc.sync.dma_start(
                        q8_sb[0:33],
                        q8[pair, 0:33].rearrange("p (i s) -> p i s", i=2))
                    nc.sync.dma_start(
                        q8_sb[64:97],
                        q8[pair, 33:66].rearrange("p (i s) -> p i s", i=2))
                    nc.scalar.dma_start(
                        k8_sb[0:33],
                        k8[pair, 0:33].rearrange("p (i s) -> p i s", i=2))
                    nc.scalar.dma_start(
                        k8_sb[64:97],
                        k8[pair, 33:66].rearrange("p (i s) -> p i s", i=2))
                nc.gpsimd.dma_start(
                    v8_sb, v8[pair].rearrange("p (h t i f) -> p h t i f",
                                              h=2, i=2, f=VF))
                return qT_sb, kT_sb, vA_sb, q8_sb, k8_sb, v8_sb
            if split_first:
                cut = SQ_BLK
                nc.sync.dma_start(kT_sb[:, :P], kT[pair, :, :P])
                nc.scalar.dma_start(qT_sb[:, :cut], qT[pair, :, :cut])
                nc.gpsimd.dma_start(
                    tmid_sb, tmc[:].rearrange("p (i f) -> p i f", i=2))
                nc.scalar.dma_start(kT_sb[:, P:cut], kT[pair, :, P:cut])
                nc.sync.dma_start(kT_sb[:, cut:], kT[pair, :, cut:])
                nc.sync.dma_start(qT_sb[:, cut:], qT[pair, :, cut:])
            else:
                nc.sync.dma_start(qT_sb, qT[pair])
                nc.scalar.dma_start(kT_sb, kT[pair])
            nc.gpsimd.dma_start(
                vA_sb, vA[pair].rearrange("p (h t f) -> p h t f", h=2, f=VF))
            nc.gpsimd.dma_start(
                q8_sb[0:33], q8[pair, 0:33].rearrange("p (i s) -> p i s", i=2))
            nc.gpsimd.dma_start(
                q8_sb[64:97],
                q8[pair, 33:66].rearrange("p (i s) -> p i s", i=2))
            nc.gpsimd.dma_start(
                k8_sb[0:33], k8[pair, 0:33].rearrange("p (i s) -> p i s", i=2))
            nc.gpsimd.dma_start(
                k8_sb[64:97],
                k8[pair, 33:66].rearrange("p (i s) -> p i s", i=2))
            nc.gpsimd.dma_start(
                v8_sb, v8[pair].rearrange("p (h t i f) -> p h t i f",
                                          h=2, i=2, f=VF))
            return qT_sb, kT_sb, vA_sb, q8_sb, k8_sb, v8_sb

        steps = _steps(cfg)
        pg = cfg.get("pair_group", 1)
        tiles_by_pair = {}
        for pr in steps[0][0]:
            tiles_by_pair[pr] = load_pair(pr, split_first=(pr == steps[0][0][0]))
        pending = None  # (pairs, j, pps) awaiting PV emission

        def emit_qk_exp(pairs, j, u_range=None, pps=None, pp8s=None,
                        pv_queue=None, pv_per_tile=0):
            """Emit QK matmuls + exp for a range of UNITS.

            Units: 0..4j-1 are full k-tiles; then "X" (diag t0, 512 cols),
            "Y" (diag t1 @0:384 + diag t3 @384:512), "Z" (diag t2 @0:256).
            pps[pair] entries: (t, o, ppt, base, slot) — the pp storage
            position of s-col s is base + (s - o); slot is the pp8 DR slot
            index for full fp8 tiles, else None.
            """
            use8 = j >= fp8_from
            s0 = j * SQ_BLK
            pps = {pair: [] for pair in pairs} if pps is None else pps
            pp8s = {} if pp8s is None else pp8s
            d0 = 4 * j                       # first diag tile index
            all_units = list(range(4 * j)) + ["X", "Y", "Z"]
            for u in (u_range if u_range is not None else all_units):
                if pv_queue:
                    for _ in range(min(pv_per_tile, len(pv_queue))):
                        pv_queue.pop(0)()
                # (tile, o, ps-col-base, ncols) sub-blocks of this unit
                if u == "X":
                    blocks = [(d0, 0, 0, SQ_BLK)]
                elif u == "Y":
                    blocks = [(d0 + 1, 128, 0, 384), (d0 + 3, 384, 384, 128)]
                elif u == "Z":
                    blocks = [(d0 + 2, 256, 0, 256)]
                else:
                    blocks = [(u, 0, 0, SQ_BLK)]
                ncols = sum(b[3] for b in blocks)
                for pair in pairs:
                    qT_sb, kT_sb, _, q8_sb, k8_sb, _ = tiles_by_pair[pair]
                    eng = sched[(pair, j, u)]
                    ps = ps_qk.tile([P, 2, SQ_BLK], f32, tag="ps")
                    for t, o, cb, n in blocks:
                        diag = t >= d0
                        k_sl = slice(t * P, (t + 1) * P)
                        for hi in (0, 1):
                            dst_ps = ps[:, hi, cb:cb + n]
                            tri_ps = ps[:, hi, cb:cb + P]
                            if use8:
                                nc.tensor.matmul(
                                    dst_ps,
                                    lhsT=k8_sb[64 * hi:64 * hi + 33, :, k_sl],
                                    rhs=q8_sb[64 * hi:64 * hi + 33, :,
                                              s0 - bq + o:s0 - bq + o + n],
                                    start=True, stop=not diag, perf_mode=DR,
                                )
                            else:
                                nc.tensor.matmul(
                                    dst_ps,
                                    lhsT=kT_sb[64 * hi:64 * hi + 64, k_sl],
                                    rhs=qT_sb[64 * hi:64 * hi + 64,
                                              s0 + o:s0 + o + n],
                                    start=True, stop=not diag,
                                )
                            if diag:
                                nc.tensor.matmul(
                                    tri_ps,
                                    lhsT=tm_sb, rhs=id_sb,
                                    start=False, stop=True,
                                )
                    # pp destination
                    if isinstance(u, int) and use8:
                        if u % 2 == 0:
                            pp8s[pair] = p8_pool.tile(
                                [P, 2, 2, SQ_BLK], f8, tag="pp8", name="pp8")
                        ppt = pp8s[pair]
                        pps[pair].append((u, 0, ppt, 0, u % 2))
                        dst = ppt[:, u % 2, :, :]
                    else:
                        dt_ = f8 if use8 else bf16
                        tag_ = "pp8d" if use8 else "pp"
                        pool_ = p8_pool if use8 else p_pool
                        ppt = pool_.tile([P, 2, SQ_BLK], dt_, tag=tag_,
                                         name=tag_)
                        for t, o, cb, n in blocks:
                            pps[pair].append((t, o, ppt, cb, None))
                        dst = ppt[:, :, 0:ncols] if ncols < SQ_BLK else ppt[:]
                    src_ps = ps[:, :, 0:ncols]
                    if u == "Y":            # non-contiguous: cols 0:512 full
                        src_ps, dst = ps[:], (ppt[:, :, :] if True else dst)
                    if eng == "act":
                        if use8:
                            nc.scalar.activation(
                                dst, src_ps, Exp, scale=1.0 / (8.0 * A8),
                                bias=bias8_sb[:, 0:1])
                        else:
                            nc.scalar.activation(
                                dst, src_ps, Exp, scale=1.0 / (8.0 * A16))
                    else:
                        if use8:
                            nc.vector.tensor_scalar(
                                dst.bitcast(i8), src_ps, D8F, 0.0, Add, Max)
                        else:
                            nc.vector.tensor_scalar(
                                dst.bitcast(i16), src_ps, B16F, 0.0, Add, Max)
            return pps

        def pv_closures(pair, pi, j, pps):
            _, _, vA_sb, _, _, v8_sb = tiles_by_pair[pair]
            use8 = j >= fp8_from
            ovtag = f"o{pi}" if not cfg.get("ov_alt") else f"o{pi}{j % 2}"
            acc = ps_ov.tile([P, 2, 4, P], f32, tag=ovtag, name="acc")
            accs = [acc[:, hi] for hi in (0, 1)]
            # build op list: (c, hi, lhsT, rhs, perf_mode)
            ops = []
            if use8:
                full = [e for e in pps if e[4] is not None]
                diag = [e for e in pps if e[4] is None]
                for fi in range(0, len(full), 2):
                    pp8 = full[fi][2]
                    tp = full[fi][0] // 2
                    for c in range(4):
                        for hi in (0, 1):
                            ops.append((c, hi,
                                        pp8[:, :, hi, c * P:(c + 1) * P],
                                        v8_sb[:, hi, tp, :, :], DR))
                for t, o, ppt, base, _ in diag:
                    tp, sl = t // 2, t % 2
                    for c in range(o // P, 4):
                        x = base + c * P - o
                        for hi in (0, 1):
                            ops.append((c, hi,
                                        ppt[:, hi, x:x + P],
                                        v8_sb[:, hi, tp, sl, :], None))
            else:
                for t, o, ppt, base, _ in pps:
                    for c in range(o // P, 4):
                        x = base + c * P - o
                        for hi in (0, 1):
                            ops.append((c, hi,
                                        ppt[:, hi, x:x + P],
                                        vA_sb[:, hi, t, :], None))
            seen = {0: False, 1: False}
            last_i = {0: None, 1: None}
            for i, (c, hi, _, _, _) in enumerate(ops):
                last_i[hi] = i
            closures = []
            for i, (c, hi, lhsT, rhs, pm) in enumerate(ops):
                def mk(i=i, c=c, hi=hi, lhsT=lhsT, rhs=rhs, pm=pm):
                    nc.tensor.matmul(
                        accs[hi][:, c, 0:VF], lhsT=lhsT, rhs=rhs,
                        start=not seen[hi], stop=(i == last_i[hi]),
                        perf_mode=pm,
                    )
                    seen[hi] = True
                closures.append(mk)

            def finish():
                out_sb = o_pool.tile([P, 2, 4, VF], bf16, tag="osb")
                if evac_sched[(pair, j, 0)] == "act":
                    nc.scalar.copy(out_sb, acc[:, :, :, 0:VF])
                else:
                    nc.vector.tensor_copy(out_sb, acc[:, :, :, 0:VF])
                nc.sync.dma_start(
                    out[pair, j].rearrange("p (h c f) -> p h c f", h=2, c=4),
                    out_sb)
            return closures, finish

        def emit_pv(pairs, j, pps):
            for pi, pair in enumerate(pairs):
                cls, fin = pv_closures(pair, pi, j, pps[pair])
                for cl in cls:
                    cl()
                fin()

        lead = cfg.get("pv_lead", 2)
        pvq = []
        for i, (pairs, j) in enumerate(steps):
            if i % NBLK == 1 and pairs[-1] + 1 < PAIRS:
                for pr in range(pairs[-1] + 1, pairs[-1] + 1 + pg):
                    tiles_by_pair[pr] = load_pair(pr)
            all_units = list(range(4 * j)) + ["X", "Y", "Z"]
            n_u = len(all_units)
            if pending is not None:
                ppairs, pj, ppps = pending
                for pi, pr in enumerate(ppairs):
                    cls, fin = pv_closures(pr, pi, pj, ppps[pr])
                    pvq.extend(cls)
                    pvq.append(fin)
            # spread pending-PV ops across QK units; leftovers roll into the
            # next step instead of draining in a burst at the seam
            n_gaps = max(1, n_u - lead + cfg.get("pv_slack", 2))
            per_tile = -(-len(pvq) // n_gaps) if pvq else 0
            pps, pp8s = {pair: [] for pair in pairs}, {}
            emit_qk_exp(pairs, j, all_units[:lead], pps, pp8s)
            emit_qk_exp(pairs, j, all_units[lead:], pps, pp8s,
                        pv_queue=pvq, pv_per_tile=per_tile)
            pending = (pairs, j, pps)
        while pvq:
            pvq.pop(0)()
        emit_pv(*pending)

    split_multiwaits(nc)
    return nc


# ---------------------------------------------------------------- host side

def _prepare(query, key, value, attention_mask):
    import ml_dtypes

    bf = ml_dtypes.bfloat16
    f8 = ml_dtypes.float8_e4m3fn
    query = np.asarray(query, dtype=np.float32)
    key = np.asarray(key, dtype=np.float32)
    value = np.asarray(value, dtype=np.float32)
    mask = np.asarray(attention_mask).astype(bool)[:, 0]   # [B, SQ, SK]

    causal = ~np.tril(np.ones((SQ, SK), dtype=bool))
    assert (mask == causal[None]).all(), "kernel specialized to causal mask"

    cfg = globals().get("KERNEL_CFG")
    cache_key = repr(("v3", cfg))
    if cache_key not in _build_cache:
        _build_cache[cache_key] = _build(cfg)
    nc = _build_cache[cache_key]

    fp8_from = (cfg or {}).get("fp8_from", FP8_FROM)
    bq = fp8_from * SQ_BLK
    bkt = 4 * fp8_from
    TP = SKT // 2

    tm = np.zeros((P, P), np.float32)
    tm[np.triu_indices(P, 1)] = TMV          # tm[s, k] = TMV if k > s
    tmid = np.concatenate(
        [tm.astype(bf), np.eye(P, dtype=bf)], axis=1)  # [P, 2*P]

    in_maps = []
    for c in range(NCORES):
        b = c // (NCORES // B)
        np_lo = (c % (NCORES // B)) * HPC
        q_c = query[:, b, np_lo:np_lo + HPC, :]          # [SQ, 8, 64]
        k_c = key[:, b, np_lo:np_lo + HPC, :]
        v_c = value[:, b, np_lo:np_lo + HPC, :]
        # bf16: [PAIRS, 128, cols]; head A h-dim on rows 0-63, head B on 64-127
        # exp-arg scale A16 folded into q.
        qT_np = np.ascontiguousarray(
            (q_c[:bq] * A16).transpose(1, 2, 0)).reshape(
            PAIRS, P, bq).astype(bf)
        kT_np = np.ascontiguousarray(
            k_c[:bkt * P].transpose(1, 2, 0)).reshape(
            PAIRS, P, bkt * P).astype(bf)
        vA_np = np.empty((PAIRS, 2, bkt, P, VF), np.float32)
        vA_np[:, :, :, :, :HN] = v_c[:bkt * P].transpose(1, 0, 2).reshape(
            PAIRS, 2, bkt, P, HN)
        vA_np[:, :, :, :, HN] = 1.0
        vA_np = np.ascontiguousarray(
            vA_np.transpose(0, 3, 1, 2, 4)).reshape(
            PAIRS, P, 2 * bkt * VF).astype(bf)
        # fp8 DR layouts: [PAIRS, 66, 2, cols]; per head 33 rows: slot-i row p
        # holds h = i*32 + p for p < 32, row 32 is the bias row (Q=1/K=BR8 in
        # slot 0, zero in slot 1). Head A rows 0-32, head B rows 33-65.
        def dr_pack(x_c, ncols, bias):
            # x_c: [ncols, 8, 64] -> [PAIRS, 66, 2, ncols]
            arr = np.zeros((PAIRS, 2, 33, 2, ncols), np.float32)
            src = x_c.reshape(ncols, PAIRS, 2, 2, 32).transpose(1, 2, 4, 3, 0)
            arr[:, :, :32] = src                       # h rows
            arr[:, :, 32, 0, :] = bias                 # bias row, slot 0
            return np.ascontiguousarray(arr.reshape(
                PAIRS, 66, 2 * ncols)).astype(f8)

        q8_np = dr_pack(q_c[bq:] * A8, SQ - bq, 1.0)
        k8_np = dr_pack(k_c, SK, BR8)
        # v8[pair][k_part, hi, tp, slot, f]
        v8_np = np.empty((PAIRS, 2, TP, 2, P, VF), np.float32)
        v8_np[:, :, :, :, :, :HN] = v_c.transpose(1, 0, 2).reshape(
            PAIRS, 2, TP, 2, P, HN)
        v8_np[:, :, :, :, :, HN] = 1.0
        v8_np = np.ascontiguousarray(
            v8_np.transpose(0, 4, 1, 2, 3, 5)).reshape(
            PAIRS, P, 2 * TP * 2 * VF).astype(f8)
        m = {"q8": q8_np, "k8": k8_np, "v8": v8_np, "tmc": tmid}
        if fp8_from > 0:
            m.update({"qT": qT_np, "kT": kT_np, "vA": vA_np})
        in_maps.append(m)
    return nc, in_maps


def _assemble(results):
    full = np.empty((SQ, B, NP * HN), np.float32)
    for c in range(NCORES):
        b = c // (NCORES // B)
        np_lo = (c % (NCORES // B)) * HPC
        o = results[c]["out"].astype(np.float32).reshape(
            PAIRS, NBLK, P, 2, 4, VF)
        # s = j*512 + cc*128 + p ; head = 2*pair + h
        o = o.transpose(1, 4, 2, 0, 3, 5).reshape(SQ, HPC, VF)
        ctx = o[:, :, :HN] / o[:, :, HN:HN + 1]
        full[:, b, np_lo * HN:(np_lo + HPC) * HN] = ctx.reshape(SQ, HPC * HN)
    return full


def _ensure_device_backend():
    from concourse._compat import axon_active

    if not axon_active():
        return
    import jax

    try:
        if len(jax.devices()) >= NCORES and jax.devices()[0].platform != "cpu":
            return
    except Exception:
        pass
    try:
        import jax.extend.backend as jeb

        jax.config.update("jax_platform_name", "")
        jeb.clear_backends()
        jax.devices()
    except Exception:
        pass


def kernel(query, key, value, attention_mask):
    from concourse.bass_utils import run_bass_kernel_spmd

    nc, in_maps = _prepare(query, key, value, attention_mask)
    _ensure_device_backend()
    res = run_bass_kernel_spmd(nc, in_maps, core_ids=list(range(NCORES)))
    return _assemble(res.results)


# revision 23
# speedup vs baseline: 1.3412x; 1.0152x over previous
"""Trainium2 Bass kernel for nn_DotProductAttention (SQ=SK=2048, B=2, NP=32, HN=64).

v3 design (8 NeuronCores, batch*heads sharded, 8 heads per core = 4 pairs):

  - S^T tiles [128 k, 2 heads, <=512 s] per (k-tile, sq-block) in PSUM.
    QK matmul: lhsT = K^T chunk (head A on partitions 0-63, head B on 64-127),
    rhs = Q^T bf16 for j<2; fp8 DoubleRow (33x2 rows incl. bias row) for j>=2.
  - The exp-arg scale is folded into Q on the host so PSUM holds
    Z = (2^m/ln2) * (S/8)  (m = mantissa bits of the prob dtype: 7 for bf16,
    3 for fp8e4m3; the fp8 path also adds +BR8 via its matmul bias row).
  - exp runs on TWO engines, statically load-balanced:
      ACT: true exp via activation(Exp, scale[, bias]) from PSUM.
      DVE: Schraudolph bit-trick: one tensor_scalar(add B, max 0) converts
           Z to int16/int8 (C-cast truncation; +0.5 folded into B) whose raw
           bits ARE bf16/fp8 probs (written through a bitcast view of the
           pp tile). max(.,0) clamps masked/underflowed scores to +0.0.
  - Causal mask: constant strictly-upper-triangular Tm (-16000) accumulated
    into diagonal-tile PSUM by one extra matmul; both exp paths then emit
    exactly 0. Diagonal tiles only compute s >= 128*t (live extent).
  - PV with pp stationary: out[128 s, 65] += pp_chunk^T @ V_aug over k-tiles;
    V_aug has a ones column so row 64 accumulates the softmax denominator.
    fp8 pp uses DoubleRow (two k-tiles per op, 0.5 cyc/col).
  - NO on-device normalize: the [128, 4, 65] accumulators are evacuated
    PSUM->SBUF as bf16 (one copy per head, on whichever exp engine the
    balancer picks) and DMA'd out; the host divides by the denominator.

The walrus build in this container only accepts ONE sync-wait per
instruction; split_multiwaits() rewrites the Tile-scheduled program.
"""

import math

import numpy as np

SQ, SK, B, NP, HN = 2048, 2048, 2, 32, 64
NCORES = 8
HPC = B * NP // NCORES          # heads per core = 8
PAIRS = HPC // 2                # 4
P = 128
SQ_BLK = 512
NBLK = SQ // SQ_BLK             # 4
SKT = SK // P                   # 16
VF = HN + 1                     # 65: V columns + ones column (denominator)
FP8_FROM = 2                    # first sq-block computed in fp8 + DoubleRow

# --- Schraudolph constants -------------------------------------------------
LN2 = math.log(2.0)
A16 = 128.0 / (8.0 * LN2)       # fold into bf16-path q: Z16 = A16 * S_raw
A8 = 8.0 / (8.0 * LN2)          # fold into fp8-path q:  Z8  = A8 * S_raw (+BR8)
BR8 = 48.0                      # fp8 matmul bias row value (exact in e4m3)
DL8 = 16.0                      # fp8 prob down-scale 2^(-DL8/8) (softmax-inv.)
C16 = 7.4                       # Schraudolph tuning constants (mean-zeroing)
C8 = 0.45
B16F = 127.0 * 128.0 + 0.5 - C16          # DVE add for int16 path
D8F = 7.0 * 8.0 + 0.5 - C8 - BR8 - DL8    # DVE add for int8 path
TMV = -16000.0                  # causal mask add (upper triangle)

_build_cache = {}


def split_multiwaits(nc):
    """Split instructions carrying >1 sem-wait into single-wait NoOp + inst."""
    import concourse.mybir as mybir

    ctr = 0
    for fn in nc.m.functions:
        for bb in fn.blocks:
            out, changed = [], False
            for inst in list(bb.instructions):
                si = inst.sync_info
                waits = list(si.on_wait) if (si is not None and si.on_wait) else []
                if len(waits) > 1:
                    for w in waits[:-1]:
                        ctr += 1
                        out.append(
                            mybir.InstNoOp(
                                name=f"splitwait-{ctr}",
                                engine=inst.engine,
                                sync_info=mybir.SyncInfo(on_wait=[w], on_update=[]),
                            )
                        )
                    si.on_wait = waits[-1:]
                    changed = True
                out.append(inst)
            if changed:
                bb.instructions = out
    return ctr


# ---------------------------------------------------------------- scheduling

# cost-model constants (ns) for the greedy two-engine exp balancer
_ACT_RATE, _ACT_FIX = 1.0 / 1.2, 185.0
_DVE_RATE, _DVE_FIX = 1.0 / 0.96, 125.0
_EVAC_N = 2 * VF * 2            # two heads' [128, 4, 65] copies, in cols


def _steps(cfg):
    if cfg.get("pair_interleave"):
        # two pairs round-robin per block: (p,j0),(p+1,j0),(p,j1),...
        seq = []
        jos = cfg.get("j_order", [[0, 1, 2, 3]] * 4)
        for base in (0, 2):
            for k in range(NBLK):
                seq.append(((base,), jos[base][k]))
                seq.append(((base + 1,), jos[base + 1][k]))
        return seq
    pg = cfg.get("pair_group", 1)
    ngroups = PAIRS // pg
    order = cfg.get("j_order",
                    [[0, 1, 2, 3]] * (ngroups - 1) + [[1, 2, 3, 0]])
    return [(tuple(range(g * pg, (g + 1) * pg)), j)
            for g in range(ngroups) for j in order[g]]


def _exp_schedule(cfg):
    """Greedy ACT/DVE assignment for exp units and acc-evac copies.

    Returns ({(pair, j, t): eng}, {(pair, j, hi): eng}, clocks).
    Each exp unit covers both heads of one k-tile: n = 2*(SQ_BLK - o) cols.
    """
    bias = cfg.get("exp_bias", {})
    clocks = {"act": 0.0, "dve": 0.0}

    def cost(e, n):
        return (n * _ACT_RATE + _ACT_FIX if e == "act"
                else n * _DVE_RATE + _DVE_FIX)

    sched, evac = {}, {}
    pat = cfg.get("alt_pattern")      # e.g. "ADADADA" cycled over units
    k = 0
    for pairs, j in _steps(cfg):
        units = [(t, 2 * SQ_BLK) for t in range(4 * j)] + [
            ("X", 2 * SQ_BLK), ("Y", 2 * SQ_BLK), ("Z", SQ_BLK)]
        for t, n in units:
            for pair in pairs:
                force = cfg.get("force_act_j0") and j == 0
                picks = {}
                for e in ("act", "dve"):
                    picks[e] = clocks[e] + cost(e, n) + bias.get(e, 0.0)
                if pat:
                    e = "act" if pat[k % len(pat)] == "A" else "dve"
                    k += 1
                else:
                    e = "act" if force else min(picks, key=picks.get)
                sched[(pair, j, t)] = e
                clocks[e] = picks[e] - bias.get(e, 0.0)
        # acc evacuation copies of the PREVIOUS step trail this step's exp;
        # approximate by charging them here.
        for pair in pairs:
            n = 2 * 4 * VF
            fe = cfg.get("force_evac")
            picks = {e: clocks[e] + cost(e, n) for e in ("act", "dve")}
            e = fe or min(picks, key=picks.get)
            evac[(pair, j, 0)] = e
            clocks[e] = picks[e]
    return sched, evac, clocks


# ---------------------------------------------------------------- build

def _build(cfg=None):
    from contextlib import ExitStack

    import concourse.bass as bass
    import concourse.tile as tile
    from concourse import mybir

    f32 = mybir.dt.float32
    bf16 = mybir.dt.bfloat16
    f8 = mybir.dt.float8e4
    i16 = mybir.dt.int16
    i8 = mybir.dt.int8
    Exp = mybir.ActivationFunctionType.Exp
    Add = mybir.AluOpType.add
    Max = mybir.AluOpType.max
    DR = mybir.MatmulPerfMode.DoubleRow

    cfg = {**{"ps_bufs": 3, "psd_bufs": 0,
              "pp_bufs": 32, "pp8_bufs": 32, "qk_bufs": 2,
              "o_bufs": 16, "ov_bufs": 1, "exp_bias": {},
              "fp8_from": FP8_FROM, "pv_first": False,
              "ov_shared": False, "force_act_j0": False,
              "alt_pattern": "DA", "pv_slack": 6,
              "pv_after_tiles": 5},
           **(cfg or {})}
    fp8_from = cfg["fp8_from"]       # first block index computed in fp8+DR
    bq = fp8_from * SQ_BLK           # bf16 q columns (s < bq), bf16 k tiles
    bkt = 4 * fp8_from               # number of bf16 k-tiles / vA tiles
    TP = SKT // 2                    # tile-pairs = 8

    sched, evac_sched, _clocks = _exp_schedule(cfg)

    nc = bass.Bass(num_devices=NCORES)
    if fp8_from > 0:
        qT = nc.dram_tensor("qT", [PAIRS, P, bq], bf16, kind="ExternalInput")
        kT = nc.dram_tensor("kT", [PAIRS, P, bkt * P], bf16,
                            kind="ExternalInput")
        vA = nc.dram_tensor("vA", [PAIRS, P, 2 * bkt * VF], bf16,
                            kind="ExternalInput")
    # 33 contraction rows per DR slot: h 0-31 plus a bias row (Q=1, K=BR8,
    # slot 1 zeroed) so PSUM holds Z8 = A8*S + BR8 directly.
    q8 = nc.dram_tensor("q8", [PAIRS, 66, 2 * (SQ - bq)], f8,
                        kind="ExternalInput")
    k8 = nc.dram_tensor("k8", [PAIRS, 66, 2 * SK], f8, kind="ExternalInput")
    v8 = nc.dram_tensor("v8", [PAIRS, P, 2 * TP * 2 * VF], f8,
                        kind="ExternalInput")
    tmc = nc.dram_tensor("tmc", [P, 2 * P], bf16, kind="ExternalInput")
    out = nc.dram_tensor("out", [PAIRS, NBLK, P, 4 * 2 * VF], bf16,
                         kind="ExternalOutput")

    with tile.TileContext(nc) as tc, ExitStack() as ctx:
        const = ctx.enter_context(tc.tile_pool(name="const", bufs=1))
        qk_pool = ctx.enter_context(tc.tile_pool(name="qk", bufs=cfg["qk_bufs"]))
        p_pool = ctx.enter_context(tc.tile_pool(name="p", bufs=cfg["pp_bufs"]))
        p8_pool = ctx.enter_context(
            tc.tile_pool(name="p8", bufs=cfg["pp8_bufs"]))
        o_pool = ctx.enter_context(tc.tile_pool(name="o", bufs=cfg["o_bufs"]))
        ps_qk = ctx.enter_context(
            tc.tile_pool(name="psqk", bufs=cfg["ps_bufs"], space="PSUM"))
        ps_dve = (ctx.enter_context(
            tc.tile_pool(name="psdve", bufs=cfg["psd_bufs"], space="PSUM"))
            if cfg["psd_bufs"] else ps_qk)
        ps_ov = ctx.enter_context(
            tc.tile_pool(name="psov", bufs=cfg["ov_bufs"], space="PSUM"))

        tmid_sb = const.tile([P, 2, P], bf16)
        tm_sb = tmid_sb[:, 0, :]
        id_sb = tmid_sb[:, 1, :]
        bias8_sb = const.tile([P, 1], f32)
        nc.vector.memset(bias8_sb, -(BR8 + DL8) / (8.0 * A8))

        def load_pair(pair, split_first=False):
            # split DMA dispatch across the SP and ACT sequencers so the fill
            # isn't serialized on one queue; each TAG keeps a fixed queue so
            # same-slot rewrites stay queue-ordered.
            if fp8_from > 0:
                qT_sb = qk_pool.tile([P, bq], bf16, tag="qT")
                kT_sb = qk_pool.tile([P, bkt * P], bf16, tag="kT")
                vA_sb = qk_pool.tile([P, 2, bkt, VF], bf16, tag="vA")
            else:
                qT_sb = kT_sb = vA_sb = None
            q8_sb = qk_pool.tile([97, 2, SQ - bq], f8, tag="q8")
            k8_sb = qk_pool.tile([97, 2, SK], f8, tag="k8")
            v8_sb = qk_pool.tile([P, 2, TP, 2, VF], f8, tag="v8")
            if fp8_from == 0:
                if split_first:
                    nc.sync.dma_start(
                        k8_sb[0:33],
                        k8[pair, 0:33].rearrange("p (i s) -> p i s", i=2))
                    nc.scalar.dma_start(
                        q8_sb[0:33],
                        q8[pair, 0:33].rearrange("p (i s) -> p i s", i=2))
                    nc.gpsimd.dma_start(
                        tmid_sb, tmc[:].rearrange("p (i f) -> p i f", i=2))
                    nc.sync.dma_start(
                        k8_sb[64:97],
                        k8[pair, 33:66].rearrange("p (i s) -> p i s", i=2))
                    nc.scalar.dma_start(
                        q8_sb[64:97],
                        q8[pair, 33:66].rearrange("p (i s) -> p i s", i=2))
                else:
                    nc.sync.dma_start(
                        q8_sb[0:33],
                        q8[pair, 0:33].rearrange("p (i s) -> p i s", i=2))
                    nc.sync.dma_start(
                        q8_sb[64:97],
                        q8[pair, 33:66].rearrange("p (i s) -> p i s", i=2))
                    nc.scalar.dma_start(
                        k8_sb[0:33],
                        k8[pair, 0:33].rearrange("p (i s) -> p i s", i=2))
                    nc.scalar.dma_start(
                        k8_sb[64:97],
                        k8[pair, 33:66].rearrange("p (i s) -> p i s", i=2))
                nc.gpsimd.dma_start(
                    v8_sb, v8[pair].rearrange("p (h t i f) -> p h t i f",
                                              h=2, i=2, f=VF))
                return qT_sb, kT_sb, vA_sb, q8_sb, k8_sb, v8_sb
            if split_first:
                cut = SQ_BLK
                nc.sync.dma_start(kT_sb[:, :P], kT[pair, :, :P])
                nc.scalar.dma_start(qT_sb[:, :cut], qT[pair, :, :cut])
                nc.gpsimd.dma_start(
                    tmid_sb, tmc[:].rearrange("p (i f) -> p i f", i=2))
                nc.scalar.dma_start(kT_sb[:, P:cut], kT[pair, :, P:cut])
                nc.sync.dma_start(kT_sb[:, cut:], kT[pair, :, cut:])
                nc.sync.dma_start(qT_sb[:, cut:], qT[pair, :, cut:])
            else:
                nc.sync.dma_start(qT_sb, qT[pair])
                nc.scalar.dma_start(kT_sb, kT[pair])
            nc.gpsimd.dma_start(
                vA_sb, vA[pair].rearrange("p (h t f) -> p h t f", h=2, f=VF))
            nc.gpsimd.dma_start(
                q8_sb[0:33], q8[pair, 0:33].rearrange("p (i s) -> p i s", i=2))
            nc.gpsimd.dma_start(
                q8_sb[64:97],
                q8[pair, 33:66].rearrange("p (i s) -> p i s", i=2))
            nc.gpsimd.dma_start(
                k8_sb[0:33], k8[pair, 0:33].rearrange("p (i s) -> p i s", i=2))
            nc.gpsimd.dma_start(
                k8_sb[64:97],
                k8[pair, 33:66].rearrange("p (i s) -> p i s", i=2))
            nc.gpsimd.dma_start(
                v8_sb, v8[pair].rearrange("p (h t i f) -> p h t i f",
                                          h=2, i=2, f=VF))
            return qT_sb, kT_sb, vA_sb, q8_sb, k8_sb, v8_sb

        steps = _steps(cfg)
        pg = cfg.get("pair_group", 1)
        first_use = {}
        for si, (prs, _) in enumerate(steps):
            for pr in prs:
                first_use.setdefault(pr, si)
        lead_steps = cfg.get("load_lead", 3)
        load_at = {}
        for pr, fu in first_use.items():
            load_at.setdefault(max(0, fu - lead_steps), []).append(pr)
        tiles_by_pair = {}
        for pr in sorted(load_at.get(0, [])):
            tiles_by_pair[pr] = load_pair(pr, split_first=(pr == steps[0][0][0]))
        pending = None  # (pairs, j, pps) awaiting PV emission

        def emit_qk_exp(pairs, j, u_range=None, pps=None, pp8s=None,
                        pv_queue=None, pv_per_tile=0):
            """Emit QK matmuls + exp for a range of UNITS.

            Units: 0..4j-1 are full k-tiles; then "X" (diag t0, 512 cols),
            "Y" (diag t1 @0:384 + diag t3 @384:512), "Z" (diag t2 @0:256).
            pps[pair] entries: (t, o, ppt, base, slot) — the pp storage
            position of s-col s is base + (s - o); slot is the pp8 DR slot
            index for full fp8 tiles, else None.
            """
            use8 = j >= fp8_from
            s0 = j * SQ_BLK
            pps = {pair: [] for pair in pairs} if pps is None else pps
            pp8s = {} if pp8s is None else pp8s
            d0 = 4 * j                       # first diag tile index
            all_units = list(range(4 * j)) + ["X", "Y", "Z"]
            for u in (u_range if u_range is not None else all_units):
                if pv_queue:
                    for _ in range(min(pv_per_tile, len(pv_queue))):
                        pv_queue.pop(0)()
                # (tile, o, ps-col-base, ncols) sub-blocks of this unit
                if u == "X":
                    blocks = [(d0, 0, 0, SQ_BLK)]
                elif u == "Y":
                    blocks = [(d0 + 1, 128, 0, 384), (d0 + 3, 384, 384, 128)]
                elif u == "Z":
                    blocks = [(d0 + 2, 256, 0, 256)]
                else:
                    blocks = [(u, 0, 0, SQ_BLK)]
                ncols = sum(b[3] for b in blocks)
                for pair in pairs:
                    qT_sb, kT_sb, _, q8_sb, k8_sb, _ = tiles_by_pair[pair]
                    eng = sched[(pair, j, u)]
                    ps = ps_qk.tile([P, 2, SQ_BLK], f32, tag="ps")
                    for t, o, cb, n in blocks:
                        diag = t >= d0
                        k_sl = slice(t * P, (t + 1) * P)
                        for hi in (0, 1):
                            dst_ps = ps[:, hi, cb:cb + n]
                            tri_ps = ps[:, hi, cb:cb + P]
                            if use8:
                                nc.tensor.matmul(
                                    dst_ps,
                                    lhsT=k8_sb[64 * hi:64 * hi + 33, :, k_sl],
                                    rhs=q8_sb[64 * hi:64 * hi + 33, :,
                                              s0 - bq + o:s0 - bq + o + n],
                                    start=True, stop=not diag, perf_mode=DR,
                                )
                            else:
                                nc.tensor.matmul(
                                    dst_ps,
                                    lhsT=kT_sb[64 * hi:64 * hi + 64, k_sl],
                                    rhs=qT_sb[64 * hi:64 * hi + 64,
                                              s0 + o:s0 + o + n],
                                    start=True, stop=not diag,
                                )
                            if diag:
                                nc.tensor.matmul(
                                    tri_ps,
                                    lhsT=tm_sb, rhs=id_sb,
                                    start=False, stop=True,
                                )
                    # pp destination
                    if isinstance(u, int) and use8:
                        if u % 2 == 0:
                            pp8s[pair] = p8_pool.tile(
                                [P, 2, 2, SQ_BLK], f8, tag="pp8", name="pp8")
                        ppt = pp8s[pair]
                        pps[pair].append((u, 0, ppt, 0, u % 2))
                        dst = ppt[:, u % 2, :, :]
                    else:
                        dt_ = f8 if use8 else bf16
                        tag_ = "pp8d" if use8 else "pp"
                        pool_ = p8_pool if use8 else p_pool
                        ppt = pool_.tile([P, 2, SQ_BLK], dt_, tag=tag_,
                                         name=tag_)
                        for t, o, cb, n in blocks:
                            pps[pair].append((t, o, ppt, cb, None))
                        dst = ppt[:, :, 0:ncols] if ncols < SQ_BLK else ppt[:]
                    src_ps = ps[:, :, 0:ncols]
                    if u == "Y":            # non-contiguous: cols 0:512 full
                        src_ps, dst = ps[:], (ppt[:, :, :] if True else dst)
                    if eng == "act":
                        if use8:
                            nc.scalar.activation(
                                dst, src_ps, Exp, scale=1.0 / (8.0 * A8),
                                bias=bias8_sb[:, 0:1])
                        else:
                            nc.scalar.activation(
                                dst, src_ps, Exp, scale=1.0 / (8.0 * A16))
                    else:
                        if use8:
                            nc.vector.tensor_scalar(
                                dst.bitcast(i8), src_ps, D8F, 0.0, Add, Max)
                        else:
                            nc.vector.tensor_scalar(
                                dst.bitcast(i16), src_ps, B16F, 0.0, Add, Max)
            return pps

        def pv_closures(pair, pi, j, pps):
            _, _, vA_sb, _, _, v8_sb = tiles_by_pair[pair]
            use8 = j >= fp8_from
            ovtag = f"o{pi}" if not cfg.get("ov_alt") else f"o{pi}{j % 2}"
            acc = ps_ov.tile([P, 2, 4, P], f32, tag=ovtag, name="acc")
            accs = [acc[:, hi] for hi in (0, 1)]
            # build op list: (c, hi, lhsT, rhs, perf_mode)
            ops = []
            if use8:
                full = [e for e in pps if e[4] is not None]
                diag = [e for e in pps if e[4] is None]
                for fi in range(0, len(full), 2):
                    pp8 = full[fi][2]
                    tp = full[fi][0] // 2
                    for c in range(4):
                        for hi in (0, 1):
                            ops.append((c, hi,
                                        pp8[:, :, hi, c * P:(c + 1) * P],
                                        v8_sb[:, hi, tp, :, :], DR))
                for t, o, ppt, base, _ in diag:
                    tp, sl = t // 2, t % 2
                    for c in range(o // P, 4):
                        x = base + c * P - o
                        for hi in (0, 1):
                            ops.append((c, hi,
                                        ppt[:, hi, x:x + P],
                                        v8_sb[:, hi, tp, sl, :], None))
            else:
                for t, o, ppt, base, _ in pps:
                    for c in range(o // P, 4):
                        x = base + c * P - o
                        for hi in (0, 1):
                            ops.append((c, hi,
                                        ppt[:, hi, x:x + P],
                                        vA_sb[:, hi, t, :], None))
            seen = {0: False, 1: False}
            last_i = {0: None, 1: None}
            for i, (c, hi, _, _, _) in enumerate(ops):
                last_i[hi] = i
            closures = []
            for i, (c, hi, lhsT, rhs, pm) in enumerate(ops):
                def mk(i=i, c=c, hi=hi, lhsT=lhsT, rhs=rhs, pm=pm):
                    nc.tensor.matmul(
                        accs[hi][:, c, 0:VF], lhsT=lhsT, rhs=rhs,
                        start=not seen[hi], stop=(i == last_i[hi]),
                        perf_mode=pm,
                    )
                    seen[hi] = True
                closures.append(mk)

            def finish():
                out_sb = o_pool.tile([P, 2, 4, VF], bf16, tag="osb")
                if evac_sched[(pair, j, 0)] == "act":
                    nc.scalar.copy(out_sb, acc[:, :, :, 0:VF])
                else:
                    nc.vector.tensor_copy(out_sb, acc[:, :, :, 0:VF])
                nc.sync.dma_start(
                    out[pair, j].rearrange("p (h c f) -> p h c f", h=2, c=4),
                    out_sb)
            return closures, finish

        def emit_pv(pairs, j, pps):
            for pi, pair in enumerate(pairs):
                cls, fin = pv_closures(pair, pi, j, pps[pair])
                for cl in cls:
                    cl()
                fin()

        lead = cfg.get("pv_lead", 2)
        pvq = []
        for i, (pairs, j) in enumerate(steps):
            for pr in sorted(load_at.get(i, [])):
                if pr not in tiles_by_pair:
                    tiles_by_pair[pr] = load_pair(pr)
            all_units = list(range(4 * j)) + ["X", "Y", "Z"]
            n_u = len(all_units)
            if pending is not None:
                ppairs, pj, ppps = pending
                for pi, pr in enumerate(ppairs):
                    cls, fin = pv_closures(pr, pi, pj, ppps[pr])
                    pvq.extend(cls)
                    pvq.append(fin)
            # spread pending-PV ops across QK units; leftovers roll into the
            # next step instead of draining in a burst at the seam
            n_gaps = max(1, n_u - lead + cfg.get("pv_slack", 2))
            per_tile = -(-len(pvq) // n_gaps) if pvq else 0
            pps, pp8s = {pair: [] for pair in pairs}, {}
            emit_qk_exp(pairs, j, all_units[:lead], pps, pp8s)
            emit_qk_exp(pairs, j, all_units[lead:], pps, pp8s,
                        pv_queue=pvq, pv_per_tile=per_tile)
            pending = (pairs, j, pps)
        # final step: emit its PV interleaved with the remaining queue;
        # ops only wait on their own exp units, so the PE tail is short
        fpairs, fj, fpps = pending
        for pi, pr in enumerate(fpairs):
            cls, fin = pv_closures(pr, pi, fj, fpps[pr])
            pvq.extend(cls)
            pvq.append(fin)
        while pvq:
            pvq.pop(0)()

    split_multiwaits(nc)
    return nc


# ---------------------------------------------------------------- host side

def _prepare(query, key, value, attention_mask):
    import ml_dtypes

    bf = ml_dtypes.bfloat16
    f8 = ml_dtypes.float8_e4m3fn
    query = np.asarray(query, dtype=np.float32)
    key = np.asarray(key, dtype=np.float32)
    value = np.asarray(value, dtype=np.float32)
    mask = np.asarray(attention_mask).astype(bool)[:, 0]   # [B, SQ, SK]

    causal = ~np.tril(np.ones((SQ, SK), dtype=bool))
    assert (mask == causal[None]).all(), "kernel specialized to causal mask"

    cfg = globals().get("KERNEL_CFG")
    cache_key = repr(("v3", cfg))
    if cache_key not in _build_cache:
        _build_cache[cache_key] = _build(cfg)
    nc = _build_cache[cache_key]

    fp8_from = (cfg or {}).get("fp8_from", FP8_FROM)
    bq = fp8_from * SQ_BLK
    bkt = 4 * fp8_from
    TP = SKT // 2

    tm = np.zeros((P, P), np.float32)
    tm[np.triu_indices(P, 1)] = TMV          # tm[s, k] = TMV if k > s
    tmid = np.concatenate(
        [tm.astype(bf), np.eye(P, dtype=bf)], axis=1)  # [P, 2*P]

    in_maps = []
    for c in range(NCORES):
        b = c // (NCORES // B)
        np_lo = (c % (NCORES // B)) * HPC
        q_c = query[:, b, np_lo:np_lo + HPC, :]          # [SQ, 8, 64]
        k_c = key[:, b, np_lo:np_lo + HPC, :]
        v_c = value[:, b, np_lo:np_lo + HPC, :]
        # bf16: [PAIRS, 128, cols]; head A h-dim on rows 0-63, head B on 64-127
        # exp-arg scale A16 folded into q.
        qT_np = np.ascontiguousarray(
            (q_c[:bq] * A16).transpose(1, 2, 0)).reshape(
            PAIRS, P, bq).astype(bf)
        kT_np = np.ascontiguousarray(
            k_c[:bkt * P].transpose(1, 2, 0)).reshape(
            PAIRS, P, bkt * P).astype(bf)
        vA_np = np.empty((PAIRS, 2, bkt, P, VF), np.float32)
        vA_np[:, :, :, :, :HN] = v_c[:bkt * P].transpose(1, 0, 2).reshape(
            PAIRS, 2, bkt, P, HN)
        vA_np[:, :, :, :, HN] = 1.0
        vA_np = np.ascontiguousarray(
            vA_np.transpose(0, 3, 1, 2, 4)).reshape(
            PAIRS, P, 2 * bkt * VF).astype(bf)
        # fp8 DR layouts: [PAIRS, 66, 2, cols]; per head 33 rows: slot-i row p
        # holds h = i*32 + p for p < 32, row 32 is the bias row (Q=1/K=BR8 in
        # slot 0, zero in slot 1). Head A rows 0-32, head B rows 33-65.
        def dr_pack(x_c, ncols, bias):
            # x_c: [ncols, 8, 64] -> [PAIRS, 66, 2, ncols]
            arr = np.zeros((PAIRS, 2, 33, 2, ncols), np.float32)
            src = x_c.reshape(ncols, PAIRS, 2, 2, 32).transpose(1, 2, 4, 3, 0)
            arr[:, :, :32] = src                       # h rows
            arr[:, :, 32, 0, :] = bias                 # bias row, slot 0
            return np.ascontiguousarray(arr.reshape(
                PAIRS, 66, 2 * ncols)).astype(f8)

        q8_np = dr_pack(q_c[bq:] * A8, SQ - bq, 1.0)
        k8_np = dr_pack(k_c, SK, BR8)
        # v8[pair][k_part, hi, tp, slot, f]
        v8_np = np.empty((PAIRS, 2, TP, 2, P, VF), np.float32)
        v8_np[:, :, :, :, :, :HN] = v_c.transpose(1, 0, 2).reshape(
            PAIRS, 2, TP, 2, P, HN)
        v8_np[:, :, :, :, :, HN] = 1.0
        v8_np = np.ascontiguousarray(
            v8_np.transpose(0, 4, 1, 2, 3, 5)).reshape(
            PAIRS, P, 2 * TP * 2 * VF).astype(f8)
        m = {"q8": q8_np, "k8": k8_np, "v8": v8_np, "tmc": tmid}
        if fp8_from > 0:
            m.update({"qT": qT_np, "kT": kT_np, "vA": vA_np})
        in_maps.append(m)
    return nc, in_maps


def _assemble(results):
    full = np.empty((SQ, B, NP * HN), np.float32)
    for c in range(NCORES):
        b = c // (NCORES // B)
        np_lo = (c % (NCORES // B)) * HPC
        o = results[c]["out"].astype(np.float32).reshape(
            PAIRS, NBLK, P, 2, 4, VF)
        # s = j*512 + cc*128 + p ; head = 2*pair + h
        o = o.transpose(1, 4, 2, 0, 3, 5).reshape(SQ, HPC, VF)
        ctx = o[:, :, :HN] / o[:, :, HN:HN + 1]
        full[:, b, np_lo * HN:(np_lo + HPC) * HN] = ctx.reshape(SQ, HPC * HN)
    return full


def _ensure_device_backend():
    from concourse._compat import axon_active

    if not axon_active():
        return
    import jax

    try:
        if len(jax.devices()) >= NCORES and jax.devices()[0].platform != "cpu":
            return
    except Exception:
        pass
    try:
        import jax.extend.backend as jeb

        jax.config.update("jax_platform_name", "")
        jeb.clear_backends()
        jax.devices()
    except Exception:
        pass


def kernel(query, key, value, attention_mask):
    from concourse.bass_utils import run_bass_kernel_spmd

    nc, in_maps = _prepare(query, key, value, attention_mask)
    _ensure_device_backend()
    res = run_bass_kernel_spmd(nc, in_maps, core_ids=list(range(NCORES)))
    return _assemble(res.results)
